# revision 1
# baseline (speedup 1.0000x reference)
"""Trainium2 Bass kernel for nn_GATPredictor (2-layer multi-head GAT + MLP).

kernel(**inputs) takes FULL unsharded numpy inputs, returns the FULL
(50000,) float32 output.  Internally: 8-way dst-node sharding (degree
round-robin), replicated weights, one AllGather of layer-1 src features,
padded per-(group,half) neighbor-slot gathers via dma_gather (int16 idx,
two half-tables), per-edge softmax with a per-core upper-bound max, and
PSUM-accumulated identity matmuls for the weighted scatter-add.
"""

import numpy as np

N = 50000
E = 800000
F_IN = 64
H = 4
C = 128
NEG = 0.2
R = 8
PER = N // R
DUM = 22
CHUNK = PER + DUM  # 6272
G = CHUNK // 128  # 49
NPOS = R * CHUNK  # 50176
HALF = NPOS // 2  # 25088
CJ = 8
NB = NPOS // 128  # 392

ROWF0 = 192  # table0 row: [h(128) | asrc(4) | pad] fp32 -> 768B
TAB1_BF16 = True
ROWF1 = 640 if TAB1_BF16 else 576  # [x1(512) | asrc(4) | pad]


# ---------------------------------------------------------------- host prep
def _prep_graph(edge_index, edge_weight):
    src = np.concatenate([np.asarray(edge_index[0], np.int64), np.arange(N)])
    dst = np.concatenate([np.asarray(edge_index[1], np.int64), np.arange(N)])
    ew = np.concatenate([np.asarray(edge_weight, np.float32), np.ones(N, np.float32)])

    deg = np.bincount(dst, minlength=N)
    gorder = np.argsort(deg, kind="stable")
    rank_of_node = np.empty(N, np.int64)
    rank_of_node[gorder] = np.arange(N) % R

    src_in_A = rank_of_node[src] < 4
    degA = np.bincount(dst[src_in_A], minlength=N)

    pos_of_node = np.empty(N, np.int64)
    perm_per_rank = []
    for r in range(R):
        nodes = np.where(rank_of_node == r)[0]
        order = np.lexsort((degA[nodes], deg[nodes]))
        sorted_nodes = nodes[order]
        perm_per_rank.append(sorted_nodes)
        pos_of_node[sorted_nodes] = CHUNK * r + DUM + np.arange(nodes.shape[0])

    src_pos = pos_of_node[src]
    dst_pos = pos_of_node[dst]
    e_half = (src_pos >= HALF).astype(np.int64)
    e_idx_in_half = np.where(e_half == 0, src_pos, src_pos - HALF)

    halfdeg = np.zeros((NPOS, 2), np.int64)
    np.add.at(halfdeg, (dst_pos, e_half), 1)
    hv = halfdeg.reshape(R, G, 128, 2)
    Duni = hv.max(axis=(0, 2))  # (G, 2)

    call_plan = []
    for g in range(G):
        for h in range(2):
            left = int(Duni[g, h])
            while left > 0:
                c = min(CJ, left)
                call_plan.append((g, h, c))
                left -= c
    tot_cols = int(Duni.sum())

    colbase = np.zeros((G, 2), np.int64)
    acc = 0
    for g in range(G):
        for h in range(2):
            colbase[g, h] = acc
            acc += int(Duni[g, h])
    assert acc == tot_cols

    order = np.lexsort((e_half, dst_pos))
    sd, sh, si, sw = dst_pos[order], e_half[order], e_idx_in_half[order], ew[order]
    key = sd * 2 + sh
    change = np.empty(key.shape[0], np.bool_)
    change[0] = True
    change[1:] = key[1:] != key[:-1]
    run_start = np.maximum.accumulate(np.where(change, np.arange(key.shape[0]), 0))
    slot_k = np.arange(key.shape[0]) - run_start

    e_rank = sd // CHUNK
    e_loc = sd - e_rank * CHUNK
    e_g = e_loc // 128
    e_p = e_loc % 128
    e_col = colbase[e_g, sh] + slot_k

    gidx = np.zeros((R, 128, tot_cols), np.int32)
    ewt = np.zeros((R, 128, tot_cols), np.float32)
    gidx[e_rank, e_p, e_col] = si
    ewt[e_rank, e_p, e_col] = sw

    idx16 = np.zeros((R, 128, 8 * tot_cols), np.int16)
    cursor = {(g, h): 0 for g in range(G) for h in range(2)}
    off = 0
    for g, h, c in call_plan:
        cb = colbase[g, h] + cursor[(g, h)]
        cursor[(g, h)] += c
        cols = gidx[:, :, cb : cb + c]  # (R, 128, c)
        stream = cols.transpose(0, 2, 1).reshape(R, -1)
        w = stream.reshape(R, c * 8, 16).transpose(0, 2, 1).astype(np.int16)
        idx16[:, :, off : off + 8 * c] = np.tile(w, (1, 8, 1))
        off += 8 * c
    assert off == 8 * tot_cols

    meta = dict(call_plan=call_plan, tot_cols=tot_cols)
    return perm_per_rank, gidx, ewt, idx16, meta


def _prep_weights(W_in, b_in, lin0, a_src0, a_dst0, lin1, a_src1, a_dst1):
    def fold(lin, a):
        return np.einsum(
            "ihc,hc->ih",
            lin.reshape(lin.shape[0], H, C).astype(np.float64),
            a.astype(np.float64),
        ).astype(np.float32)

    w_src0, w_dst0 = fold(lin0, a_src0), fold(lin0, a_dst0)
    rhs0 = np.zeros((128, 136), np.float32)
    rhs0[:F_IN, :C] = W_in
    rhs0[:F_IN, C : C + 4] = W_in @ w_src0
    rhs0[:F_IN, C + 4 :] = W_in @ w_dst0
    rhs0[F_IN, :C] = b_in
    rhs0[F_IN, C : C + 4] = b_in @ w_src0
    rhs0[F_IN, C + 4 :] = b_in @ w_dst0
    rhs0L = np.zeros((128, 132), np.float32)
    rhs0L[:F_IN, :C] = W_in
    rhs0L[:F_IN, C:] = W_in @ w_dst0
    rhs0L[F_IN, :C] = b_in
    rhs0L[F_IN, C:] = b_in @ w_dst0
    wa1 = np.concatenate([fold(lin1, a_src1), fold(lin1, a_dst1)], axis=1)
    return rhs0, rhs0L, wa1.astype(np.float32)


def _build_xpt_blocks(X, perm_per_rank):
    xpt = np.zeros((128, NPOS), np.float32)
    xpt[F_IN, :] = 1.0
    for r in range(R):
        cols = CHUNK * r + DUM + np.arange(perm_per_rank[r].shape[0])
        xpt[:F_IN, cols] = X[perm_per_rank[r]].T
    return xpt.reshape(128, NB, 128).transpose(1, 0, 2).copy()


# ---------------------------------------------------------------- bass build
def _build_nc(call_plan, tot_cols, dbg=False):
    import concourse.bacc as bacc
    import concourse.bass_isa as bass_isa
    import concourse.mybir as mybir
    import concourse.tile as tile
    from concourse.masks import make_identity

    f32 = mybir.dt.float32
    bf16 = mybir.dt.bfloat16
    i16 = mybir.dt.int16
    tdt = bf16 if TAB1_BF16 else f32
    AF = mybir.ActivationFunctionType
    OP = mybir.AluOpType
    AX = mybir.AxisListType

    nc = bacc.Bacc(
        "TRN2",
        target_bir_lowering=False,
        debug=False,
        enable_asserts=False,
        num_devices=R,
        num_swdge_queues=4,
    )

    xptb = nc.dram_tensor("xptb", [NB, 128, 128], f32, kind="ExternalInput")
    xptl = nc.dram_tensor("xptl", [G, 128, 128], f32, kind="ExternalInput")
    rhs0_d = nc.dram_tensor("rhs0", [128, 136], f32, kind="ExternalInput")
    rhs0l_d = nc.dram_tensor("rhs0l", [128, 132], f32, kind="ExternalInput")
    lin0_d = nc.dram_tensor("lin0", [128, 512], f32, kind="ExternalInput")
    lin1_d = nc.dram_tensor("lin1", [512, 512], f32, kind="ExternalInput")
    wa1_d = nc.dram_tensor("wa1", [512, 8], f32, kind="ExternalInput")
    wp1_d = nc.dram_tensor("wp1", [640, 128], f32, kind="ExternalInput")
    brow_d = nc.dram_tensor("brow", [1, 1280], f32, kind="ExternalInput")
    bp2_d = nc.dram_tensor("bp2", [1, 1], f32, kind="ExternalInput")
    idx_d = nc.dram_tensor("idx16", [128, 8 * tot_cols], i16, kind="ExternalInput")
    ew_d = nc.dram_tensor("ewt", [128, tot_cols], f32, kind="ExternalInput")
    y_d = nc.dram_tensor("y", [128, G], f32, kind="ExternalOutput")
    if dbg:
        dbg_tab0 = nc.dram_tensor("dbg_tab0", [128, ROWF0], f32, kind="ExternalOutput")
        dbg_xg = nc.dram_tensor("dbg_xg", [128, CJ * ROWF0], f32, kind="ExternalOutput")
        dbg_et = nc.dram_tensor("dbg_et", [128, 8 * CJ], f32, kind="ExternalOutput")
        dbg_m = nc.dram_tensor("dbg_m", [128, 24], f32, kind="ExternalOutput")
        dbg_adst0 = nc.dram_tensor("dbg_adst0", [128, 4 * G], f32, kind="ExternalOutput")
        dbg_sacc = nc.dram_tensor("dbg_sacc", [128, 4 * G], f32, kind="ExternalOutput")
        dbg_h1 = nc.dram_tensor("dbg_h1", [128, 512], f32, kind="ExternalOutput")
        dbg_agin = nc.dram_tensor("dbg_agin", [128, ROWF1], f32, kind="ExternalOutput")
        dbg_h2 = nc.dram_tensor("dbg_h2", [128, 512], f32, kind="ExternalOutput")
        dbg_acc = nc.dram_tensor("dbg_acc", [128, 512], f32, kind="ExternalOutput")
        dbg_out0 = nc.dram_tensor("dbg_out0", [128, 512], f32, kind="ExternalOutput")
        dbg_wx = nc.dram_tensor("dbg_wx", [128, 512], f32, kind="ExternalOutput")

    # group -> list of (idx_off8, col, ncols, half); order mirrors host prep
    plan_by_group = [[] for _ in range(G)]
    colbase = {}
    acc = 0
    for g in range(G):
        for h in range(2):
            cols_gh = sum(c for (gg, hh, c) in call_plan if gg == g and hh == h)
            colbase[(g, h)] = acc
            acc += cols_gh
    cursor = {(g, h): 0 for g in range(G) for h in range(2)}
    off8 = 0
    for g, h, c in call_plan:
        col = colbase[(g, h)] + cursor[(g, h)]
        cursor[(g, h)] += c
        plan_by_group[g].append((off8, col, c, h))
        off8 += 8 * c

    with tile.TileContext(nc) as tc:
        with tc.tile_pool(name="dram", bufs=1, space="DRAM") as dram, tc.tile_pool(
            name="const", bufs=1
        ) as cp:
            table0 = dram.tile([NPOS, ROWF0], f32)
            h_res_d = dram.tile([G, 128, 128], f32)
            agin = dram.tile([CHUNK, ROWF1], tdt)
            table1 = dram.tile([NPOS, ROWF1], tdt)

            ident = cp.tile([128, 128], f32)
            make_identity(nc, ident[:])
            identb = cp.tile([128, 128], bf16)
            nc.vector.tensor_copy(out=identb[:], in_=ident[:])
            rhs0_s = cp.tile([128, 136], f32)
            nc.sync.dma_start(out=rhs0_s[:], in_=rhs0_d[:, :])
            rhs0l_s = cp.tile([128, 132], f32)
            nc.sync.dma_start(out=rhs0l_s[:], in_=rhs0l_d[:, :])
            lin0_s = cp.tile([128, 512], f32)
            nc.sync.dma_start(out=lin0_s[:], in_=lin0_d[:, :])
            lin1_s = [cp.tile([128, 512], f32, tag=f"lin1_{c}", name=f"lin1s{c}") for c in range(4)]
            wa1_s = [cp.tile([128, 8], f32, tag=f"wa1_{c}", name=f"wa1s{c}") for c in range(4)]
            for c in range(4):
                nc.sync.dma_start(
                    out=lin1_s[c][:], in_=lin1_d[128 * c : 128 * (c + 1), :]
                )
                nc.sync.dma_start(
                    out=wa1_s[c][:], in_=wa1_d[128 * c : 128 * (c + 1), :]
                )
            wp1_s = [cp.tile([128, 128], f32, tag=f"wp1_{c}", name=f"wp1s{c}") for c in range(5)]
            for c in range(5):
                nc.sync.dma_start(
                    out=wp1_s[c][:], in_=wp1_d[128 * c : 128 * (c + 1), :]
                )
            brow = cp.tile([128, 1280], f32)
            nc.sync.dma_start(out=brow[0:1, :], in_=brow_d[:, :])
            nc.gpsimd.partition_broadcast(brow[:], brow[0:1, :])
            bias0r = brow[:, 0:512]
            bias1r = brow[:, 512:1024]
            bp1r = brow[:, 1024:1152]
            wp2r = brow[:, 1152:1280]
            bp2t = cp.tile([128, 1], f32)
            nc.sync.dma_start(out=bp2t[0:1, :], in_=bp2_d[:, :])
            nc.gpsimd.partition_broadcast(bp2t[:], bp2t[0:1, :])
            idx_s = cp.tile([128, 8 * tot_cols], i16)
            nc.sync.dma_start(out=idx_s[:], in_=idx_d[:, :])
            ew_s = cp.tile([128, tot_cols], f32)
            nc.sync.dma_start(out=ew_s[:], in_=ew_d[:, :])
            adst0_s = cp.tile([128, G * 4], f32)
            adst1_s = cp.tile([128, G * 4], f32)
            ysb = cp.tile([128, G], f32)
            m0t = cp.tile([128, 4], f32)
            m1t = cp.tile([128, 4], f32)
            amax = cp.tile([128, 16], f32)
            nc.vector.memset(amax[:], -1e30)
            sacc_all = cp.tile([128, 4 * G], f32)

            # ---------------- M0: replicated table0 + M0L local
            with nc.named_scope("m0"), tc.tile_pool(name="m0s", bufs=3) as mp, tc.tile_pool(
                name="m0p", bufs=2, space="PSUM"
            ) as mpp:
                for b in range(NB):
                    xb = mp.tile([128, 128], f32, tag="xb")
                    nc.sync.dma_start(out=xb[:], in_=xptb[b, :, :])
                    ps = mpp.tile([128, 136], f32, tag="m0ps")
                    nc.tensor.matmul(
                        ps[:], lhsT=xb[:], rhs=rhs0_s[:], start=True, stop=True
                    )
                    stg = mp.tile([128, ROWF0], f32, tag="stg0")
                    nc.scalar.copy(out=stg[:, 0:132], in_=ps[:, 0:132])
                    nc.vector.tensor_tensor(
                        out=amax[:, 0:4], in0=amax[:, 0:4], in1=ps[:, 128:132], op=OP.max
                    )
                    if b % G == 0:
                        nc.vector.memset(stg[0:1, 128:132], -1e30)
                    nc.sync.dma_start(
                        out=table0[128 * b : 128 * (b + 1), :], in_=stg[:]
                    )
                for g in range(G):
                    xb = mp.tile([128, 128], f32, tag="xb")
                    nc.sync.dma_start(out=xb[:], in_=xptl[g, :, :])
                    ps = mpp.tile([128, 136], f32, tag="m0ps")
                    nc.tensor.matmul(
                        ps[:, 0:132], lhsT=xb[:], rhs=rhs0l_s[:], start=True, stop=True
                    )
                    stg = mp.tile([128, 128], f32, tag="stgL")
                    nc.scalar.copy(out=stg[:], in_=ps[:, 0:128])
                    nc.sync.dma_start(out=h_res_d[g, :, :], in_=stg[:])
                    nc.vector.tensor_copy(
                        out=adst0_s[:, 4 * g : 4 * (g + 1)], in_=ps[:, 128:132]
                    )
                    nc.vector.tensor_tensor(
                        out=amax[:, 4:8], in0=amax[:, 4:8], in1=ps[:, 128:132], op=OP.max
                    )

            nc.gpsimd.partition_all_reduce(
                amax[:, 0:8], amax[:, 0:8], 128, bass_isa.ReduceOp.max
            )
            nc.vector.tensor_tensor(
                out=m0t[:], in0=amax[:, 0:4], in1=amax[:, 4:8], op=OP.add
            )
            nc.vector.tensor_scalar(
                out=m1t[:], in0=m0t[:], scalar1=NEG, scalar2=None, op0=OP.mult
            )
            nc.vector.tensor_tensor(out=m0t[:], in0=m0t[:], in1=m1t[:], op=OP.max)
            if dbg:
                nc.sync.dma_start(out=dbg_tab0[:, :], in_=table0[0:128, :])
                nc.sync.dma_start(out=dbg_m[:, 0:4], in_=m0t[:])
                nc.sync.dma_start(out=dbg_adst0[:, :], in_=adst0_s[:])

            qn = [0]

            def gather(out_ap, half_ap, ioff, c, elem):
                nc.gpsimd.dma_gather(
                    out_ap=out_ap,
                    in_ap=half_ap,
                    idxs_ap=idx_s[:, ioff : ioff + 8 * c],
                    num_idxs=128 * c,
                    num_idxs_reg=128 * c,
                    elem_size=elem,
                    single_packet=False,
                    queue_num=qn[0] % 4,
                )
                qn[0] += 1

            # ---------------- L0 aggregation + M2 (fused per group)
            with nc.named_scope("l0"), tc.tile_pool(name="l0g", bufs=5) as gp0, tc.tile_pool(
                name="l0w", bufs=6
            ) as wp0, tc.tile_pool(name="l0e", bufs=8) as ep0, tc.tile_pool(
                name="l0m", bufs=2
            ) as sp0, tc.tile_pool(
                name="l0acc", bufs=3, space="PSUM"
            ) as accp0, tc.tile_pool(
                name="l0out", bufs=1, space="PSUM"
            ) as outp0, tc.tile_pool(
                name="l0a", bufs=1, space="PSUM"
            ) as ap0, tc.tile_pool(
                name="l0t", bufs=2, space="PSUM"
            ) as trp0:
                for g in range(G):
                    calls = plan_by_group[g]
                    ncols = sum(c for (_, _, c, _) in calls)
                    accps = accp0.tile([128, 512], f32, tag="acc")
                    s_acc = sp0.tile([128, 4], f32, tag="sacc")
                    nc.vector.memset(s_acc[:], 0.0)
                    coli = 0
                    for ioff, col, c, h in calls:
                        xg = gp0.tile([128, CJ, ROWF0], f32, tag="xg")
                        gather(
                            xg[:, 0:c, :],
                            table0[0:HALF, :] if h == 0 else table0[HALF:NPOS, :],
                            ioff,
                            c,
                            ROWF0,
                        )
                        et = ep0.tile([128, 4, CJ], f32, tag="et")
                        nc.vector.tensor_tensor(
                            out=et[:, :, 0:c],
                            in0=xg[:, 0:c, 128:132].rearrange("p c f -> p f c"),
                            in1=adst0_s[:, 4 * g : 4 * (g + 1)].to_broadcast(
                                [128, 4, c]
                            ),
                            op=OP.add,
                        )
                        lr = ep0.tile([128, 4, CJ], f32, tag="lr")
                        nc.vector.tensor_scalar(
                            out=lr[:, :, 0:c], in0=et[:, :, 0:c], scalar1=NEG,
                            scalar2=None, op0=OP.mult,
                        )
                        nc.vector.tensor_tensor(
                            out=et[:, :, 0:c], in0=et[:, :, 0:c], in1=lr[:, :, 0:c],
                            op=OP.max,
                        )
                        nc.vector.tensor_tensor(
                            out=et[:, :, 0:c],
                            in0=et[:, :, 0:c],
                            in1=m0t[:].to_broadcast([128, 4, c]),
                            op=OP.subtract,
                        )
                        nc.scalar.activation(et[:, :, 0:c], et[:, :, 0:c], AF.Exp)
                        if dbg and g == 0 and ioff == 0:
                            nc.sync.dma_start(out=dbg_xg[:, :], in_=xg[:].rearrange("p a b -> p (a b)"))
                            nc.sync.dma_start(out=dbg_et[:, 0:4*CJ], in_=et[:].rearrange("p a b -> p (a b)"))
                        red = ep0.tile([128, 4], f32, tag="red")
                        nc.vector.tensor_reduce(
                            red[:], et[:, :, 0:c], axis=AX.X, op=OP.add
                        )
                        nc.vector.tensor_tensor(
                            out=s_acc[:], in0=s_acc[:], in1=red[:], op=OP.add
                        )
                        wt = ep0.tile([128, 4, CJ], f32, tag="wt")
                        nc.vector.tensor_tensor(
                            out=wt[:, :, 0:c],
                            in0=et[:, :, 0:c],
                            in1=ew_s[:, None, col : col + c].to_broadcast([128, 4, c]),
                            op=OP.mult,
                        )
                        for k in range(c):
                            wx = wp0.tile([128, 512], bf16, tag="wx")
                            nc.vector.tensor_tensor(
                                out=wx[:].rearrange("p (h f) -> p h f", h=4),
                                in0=xg[:, k, 0:128][:, None, :].to_broadcast(
                                    [128, 4, 128]
                                ),
                                in1=wt[:, :, k].to_broadcast([128, 4, 128]),
                                op=OP.mult,
                            )
                            if dbg and g == 0 and coli == 0:
                                nc.sync.dma_start(out=dbg_wx[:, :], in_=wx[:])
                            nc.tensor.matmul(
                                accps[:],
                                lhsT=identb[:],
                                rhs=wx[:],
                                start=(coli == 0),
                                stop=(coli == ncols - 1),
                            )
                            coli += 1
                    if dbg:
                        nc.vector.tensor_copy(out=sacc_all[:, 4*g:4*(g+1)], in_=s_acc[:])
                    srec = sp0.tile([128, 4], f32, tag="srec")
                    nc.vector.tensor_scalar(
                        out=srec[:], in0=s_acc[:], scalar1=1e-16, scalar2=None, op0=OP.add
                    )
                    nc.vector.reciprocal(srec[:], srec[:])
                    acc_sb = sp0.tile([128, 512], f32, tag="accsb")
                    nc.scalar.copy(out=acc_sb[:], in_=accps[:])
                    if dbg and g == 0:
                        nc.sync.dma_start(out=dbg_acc[:, :], in_=acc_sb[:])
                    aggT = sp0.tile([128, 512], f32, tag="aggT")
                    for h in range(4):
                        trp = trp0.tile([128, 128], f32, tag="tr", name=f"tr0_{g}_{h}")
                        nc.tensor.transpose(
                            out=trp[:],
                            in_=acc_sb[:, 128 * h : 128 * (h + 1)],
                            identity=ident[:],
                        )
                        nc.scalar.copy(
                            out=aggT[:, 128 * h : 128 * (h + 1)], in_=trp[:]
                        )
                    outps = outp0.tile([128, 512], f32, tag="out0")
                    for h in range(4):
                        nc.tensor.matmul(
                            outps[:, 128 * h : 128 * (h + 1)],
                            lhsT=aggT[:, 128 * h : 128 * (h + 1)],
                            rhs=lin0_s[:, 128 * h : 128 * (h + 1)],
                            start=True,
                            stop=True,
                        )
                    h1 = sp0.tile([128, 512], f32, tag="h1")
                    if dbg and g == 0:
                        nc.vector.tensor_copy(out=h1[:], in_=outps[:])
                        nc.sync.dma_start(out=dbg_out0[:, :], in_=h1[:])
                    for h in range(4):
                        nc.scalar.activation(
                            h1[:, 128 * h : 128 * (h + 1)],
                            outps[:, 128 * h : 128 * (h + 1)],
                            AF.Copy,
                            scale=srec[:, h : h + 1],
                        )
                    nc.vector.tensor_tensor(out=h1[:], in0=h1[:], in1=bias0r, op=OP.add)
                    u = sp0.tile([128, 512], f32, tag="elu_u")
                    nc.vector.tensor_scalar(
                        out=u[:], in0=h1[:], scalar1=0.0, scalar2=None, op0=OP.min
                    )
                    v = sp0.tile([128, 512], f32, tag="elu_v")
                    nc.scalar.activation(v[:], u[:], AF.Exp)
                    nc.vector.tensor_tensor(out=h1[:], in0=h1[:], in1=u[:], op=OP.subtract)
                    nc.vector.tensor_tensor(out=h1[:], in0=h1[:], in1=v[:], op=OP.add)
                    nc.vector.tensor_scalar(
                        out=h1[:], in0=h1[:], scalar1=1.0, scalar2=None, op0=OP.subtract
                    )
                    h1T = sp0.tile([128, 512], f32, tag="h1T")
                    for cc in range(4):
                        trp = trp0.tile([128, 128], f32, tag="tr", name=f"trh1_{g}_{cc}")
                        nc.tensor.transpose(
                            out=trp[:],
                            in_=h1[:, 128 * cc : 128 * (cc + 1)],
                            identity=ident[:],
                        )
                        nc.scalar.copy(
                            out=h1T[:, 128 * cc : 128 * (cc + 1)], in_=trp[:]
                        )
                    x1ps = outp0.tile([128, 512], f32, tag="x1")
                    aps = ap0.tile([128, 8], f32, tag="aps")
                    for cc in range(4):
                        nc.tensor.matmul(
                            x1ps[:],
                            lhsT=h1T[:, 128 * cc : 128 * (cc + 1)],
                            rhs=lin1_s[cc][:],
                            start=(cc == 0),
                            stop=(cc == 3),
                        )
                    for cc in range(4):
                        nc.tensor.matmul(
                            aps[:],
                            lhsT=h1T[:, 128 * cc : 128 * (cc + 1)],
                            rhs=wa1_s[cc][:],
                            start=(cc == 0),
                            stop=(cc == 3),
                        )
                    stg = sp0.tile([128, ROWF1], tdt, tag="stg1")
                    nc.scalar.copy(out=stg[:, 0:512], in_=x1ps[:])
                    nc.vector.tensor_copy(out=stg[:, 512:516], in_=aps[:, 0:4])
                    nc.vector.tensor_copy(
                        out=adst1_s[:, 4 * g : 4 * (g + 1)], in_=aps[:, 4:8]
                    )
                    nc.vector.tensor_tensor(
                        out=amax[:, 8:12], in0=amax[:, 8:12], in1=aps[:, 0:4], op=OP.max
                    )
                    nc.vector.tensor_tensor(
                        out=amax[:, 12:16], in0=amax[:, 12:16], in1=aps[:, 4:8], op=OP.max
                    )
                    if g == 0:
                        nc.vector.memset(stg[0:1, 512:516], -1e30)
                    if dbg and g == 0:
                        nc.sync.dma_start(out=dbg_h1[:, :], in_=h1[:])
                    nc.sync.dma_start(
                        out=agin[128 * g : 128 * (g + 1), :], in_=stg[:]
                    )
                    if dbg and g == 0:
                        nc.sync.dma_start(out=dbg_agin[:, :], in_=agin[0:128, :])

            nc.gpsimd.partition_all_reduce(
                amax[:, 8:16], amax[:, 8:16], 128, bass_isa.ReduceOp.max
            )
            nc.vector.tensor_tensor(
                out=m1t[:], in0=amax[:, 8:12], in1=amax[:, 12:16], op=OP.add
            )
            mtmp = cp.tile([128, 4], f32)
            nc.vector.tensor_scalar(
                out=mtmp[:], in0=m1t[:], scalar1=NEG, scalar2=None, op0=OP.mult
            )
            nc.vector.tensor_tensor(out=m1t[:], in0=m1t[:], in1=mtmp[:], op=OP.max)
            if dbg:
                nc.sync.dma_start(out=dbg_m[:, 4:8], in_=m1t[:])
                nc.sync.dma_start(out=dbg_m[:, 8:24], in_=amax[:])
                nc.sync.dma_start(out=dbg_sacc[:, :], in_=sacc_all[:])

            # ---------------- AllGather table1
            with nc.named_scope("ag"):
                nc.gpsimd.collective_compute(
                    "AllGather",
                    mybir.AluOpType.bypass,
                    replica_groups=[list(range(R))],
                    ins=[agin[:].opt()],
                    outs=[table1[:].opt()],
                )

            # ---------------- L1 aggregation + final MLP (fused per group)
            with nc.named_scope("l1"), tc.tile_pool(name="l1g", bufs=4) as gp1, tc.tile_pool(
                name="l1w", bufs=6
            ) as wp1p, tc.tile_pool(name="l1e", bufs=8) as ep1, tc.tile_pool(
                name="l1m", bufs=2
            ) as sp1, tc.tile_pool(
                name="l1acc", bufs=4, space="PSUM"
            ) as accp1, tc.tile_pool(
                name="l1z", bufs=2, space="PSUM"
            ) as zp1, tc.tile_pool(
                name="l1t", bufs=2, space="PSUM"
            ) as trp1:
                for g in range(G):
                    calls = plan_by_group[g]
                    ncols = sum(c for (_, _, c, _) in calls)
                    accps = accp1.tile([128, 512], f32, tag="acc")
                    s_acc = sp1.tile([128, 4], f32, tag="sacc")
                    nc.vector.memset(s_acc[:], 0.0)
                    coli = 0
                    for ioff, col, c, h in calls:
                        xg = gp1.tile([128, CJ, ROWF1], tdt, tag="xg")
                        gather(
                            xg[:, 0:c, :],
                            table1[0:HALF, :] if h == 0 else table1[HALF:NPOS, :],
                            ioff,
                            c,
                            ROWF1,
                        )
                        et = ep1.tile([128, 4, CJ], f32, tag="et")
                        nc.vector.tensor_tensor(
                            out=et[:, :, 0:c],
                            in0=xg[:, 0:c, 512:516].rearrange("p c f -> p f c"),
                            in1=adst1_s[:, 4 * g : 4 * (g + 1)].to_broadcast(
                                [128, 4, c]
                            ),
                            op=OP.add,
                        )
                        lr = ep1.tile([128, 4, CJ], f32, tag="lr")
                        nc.vector.tensor_scalar(
                            out=lr[:, :, 0:c], in0=et[:, :, 0:c], scalar1=NEG,
                            scalar2=None, op0=OP.mult,
                        )
                        nc.vector.tensor_tensor(
                            out=et[:, :, 0:c], in0=et[:, :, 0:c], in1=lr[:, :, 0:c],
                            op=OP.max,
                        )
                        nc.vector.tensor_tensor(
                            out=et[:, :, 0:c],
                            in0=et[:, :, 0:c],
                            in1=m1t[:].to_broadcast([128, 4, c]),
                            op=OP.subtract,
                        )
                        nc.scalar.activation(et[:, :, 0:c], et[:, :, 0:c], AF.Exp)
                        red = ep1.tile([128, 4], f32, tag="red")
                        nc.vector.tensor_reduce(
                            red[:], et[:, :, 0:c], axis=AX.X, op=OP.add
                        )
                        nc.vector.tensor_tensor(
                            out=s_acc[:], in0=s_acc[:], in1=red[:], op=OP.add
                        )
                        wt = ep1.tile([128, 4, CJ], tdt, tag="wt")
                        nc.vector.tensor_tensor(
                            out=wt[:, :, 0:c],
                            in0=et[:, :, 0:c],
                            in1=ew_s[:, None, col : col + c].to_broadcast([128, 4, c]),
                            op=OP.mult,
                        )
                        for k in range(c):
                            wx = wp1p.tile([128, 512], tdt, tag="wx")
                            nc.vector.tensor_tensor(
                                out=wx[:].rearrange("p (h f) -> p h f", h=4),
                                in0=xg[:, k, 0:512].rearrange("p (h f) -> p h f", h=4),
                                in1=wt[:, :, k].to_broadcast([128, 4, 128]),
                                op=OP.mult,
                            )
                            nc.tensor.matmul(
                                accps[:],
                                lhsT=identb[:] if TAB1_BF16 else ident[:],
                                rhs=wx[:],
                                start=(coli == 0),
                                stop=(coli == ncols - 1),
                            )
                            coli += 1
                    srec = sp1.tile([128, 4], f32, tag="srec")
                    nc.vector.tensor_scalar(
                        out=srec[:], in0=s_acc[:], scalar1=1e-16, scalar2=None, op0=OP.add
                    )
                    nc.vector.reciprocal(srec[:], srec[:])
                    h2 = sp1.tile([128, 512], f32, tag="h2")
                    for h in range(4):
                        nc.scalar.activation(
                            h2[:, 128 * h : 128 * (h + 1)],
                            accps[:, 128 * h : 128 * (h + 1)],
                            AF.Copy,
                            scale=srec[:, h : h + 1],
                        )
                    nc.vector.tensor_tensor(out=h2[:], in0=h2[:], in1=bias1r, op=OP.add)
                    u = sp1.tile([128, 512], f32, tag="elu_u")
                    nc.vector.tensor_scalar(
                        out=u[:], in0=h2[:], scalar1=0.0, scalar2=None, op0=OP.min
                    )
                    v = sp1.tile([128, 512], f32, tag="elu_v")
                    nc.scalar.activation(v[:], u[:], AF.Exp)
                    nc.vector.tensor_tensor(out=h2[:], in0=h2[:], in1=u[:], op=OP.subtract)
                    nc.vector.tensor_tensor(out=h2[:], in0=h2[:], in1=v[:], op=OP.add)
                    nc.vector.tensor_scalar(
                        out=h2[:], in0=h2[:], scalar1=1.0, scalar2=None, op0=OP.subtract
                    )
                    if dbg and g == 0:
                        nc.sync.dma_start(out=dbg_h2[:, :], in_=h2[:])
                    hT = sp1.tile([128, 640], f32, tag="hT")
                    for cc in range(4):
                        trp = trp1.tile([128, 128], f32, tag="tr", name=f"trh2_{g}_{cc}")
                        nc.tensor.transpose(
                            out=trp[:],
                            in_=h2[:, 128 * cc : 128 * (cc + 1)],
                            identity=ident[:],
                        )
                        nc.scalar.copy(
                            out=hT[:, 128 * cc : 128 * (cc + 1)], in_=trp[:]
                        )
                    hrb = sp1.tile([128, 128], f32, tag="hrb")
                    nc.sync.dma_start(out=hrb[:], in_=h_res_d[g, :, :])
                    trp = trp1.tile([128, 128], f32, tag="tr", name=f"trhr_{g}")
                    nc.tensor.transpose(out=trp[:], in_=hrb[:], identity=ident[:])
                    nc.scalar.copy(out=hT[:, 512:640], in_=trp[:])
                    zps = zp1.tile([128, 128], f32, tag="z")
                    for cc in range(5):
                        nc.tensor.matmul(
                            zps[:],
                            lhsT=hT[:, 128 * cc : 128 * (cc + 1)],
                            rhs=wp1_s[cc][:],
                            start=(cc == 0),
                            stop=(cc == 4),
                        )
                    zsb = sp1.tile([128, 128], f32, tag="zsb")
                    nc.scalar.copy(out=zsb[:], in_=zps[:])
                    nc.vector.tensor_tensor(out=zsb[:], in0=zsb[:], in1=bp1r, op=OP.add)
                    nc.scalar.activation(zsb[:], zsb[:], AF.Gelu_apprx_tanh)
                    nc.vector.tensor_tensor(out=zsb[:], in0=zsb[:], in1=wp2r, op=OP.mult)
                    yred = sp1.tile([128, 1], f32, tag="yred")
                    nc.vector.tensor_reduce(yred[:], zsb[:], axis=AX.X, op=OP.add)
                    nc.vector.tensor_scalar(
                        out=ysb[:, g : g + 1],
                        in0=yred[:],
                        scalar1=bp2t[:, 0:1],
                        scalar2=None,
                        op0=OP.add,
                    )
            nc.sync.dma_start(out=y_d[:, :], in_=ysb[:])

    nc.compile()
    return nc


_CACHE = {}
DEBUG_TAPS = False


def kernel(
    X,
    edge_index,
    edge_weight,
    W_in,
    b_in,
    lin0,
    att_src0,
    att_dst0,
    bias0,
    lin1,
    att_src1,
    att_dst1,
    bias1,
    Wp1,
    bp1,
    Wp2,
    bp2,
):
    from concourse.bass_utils import run_bass_kernel_spmd

    X = np.asarray(X, np.float32)
    perm, gidx, ewt, idx16, meta = _prep_graph(
        np.asarray(edge_index), np.asarray(edge_weight)
    )
    rhs0, rhs0l, wa1 = _prep_weights(
        np.asarray(W_in, np.float32),
        np.asarray(b_in, np.float32),
        np.asarray(lin0, np.float32),
        np.asarray(att_src0, np.float32),
        np.asarray(att_dst0, np.float32),
        np.asarray(lin1, np.float32),
        np.asarray(att_src1, np.float32),
        np.asarray(att_dst1, np.float32),
    )
    xptb = _build_xpt_blocks(X, perm)

    key = ("nc", meta["tot_cols"], tuple(meta["call_plan"]), DEBUG_TAPS)
    if key not in _CACHE:
        _CACHE[key] = _build_nc(meta["call_plan"], meta["tot_cols"], dbg=DEBUG_TAPS)
    nc = _CACHE[key]

    brow = np.zeros((1, 1280), np.float32)
    brow[0, 0:512] = np.asarray(bias0, np.float32)
    brow[0, 512:1024] = np.asarray(bias1, np.float32)
    brow[0, 1024:1152] = np.asarray(bp1, np.float32)
    brow[0, 1152:1280] = np.asarray(Wp2, np.float32)[:, 0]

    common = dict(
        xptb=xptb,
        rhs0=rhs0,
        rhs0l=rhs0l,
        lin0=np.ascontiguousarray(np.asarray(lin0, np.float32)),
        lin1=np.ascontiguousarray(np.asarray(lin1, np.float32)),
        wa1=wa1,
        wp1=np.ascontiguousarray(np.asarray(Wp1, np.float32)),
        brow=brow,
        bp2=np.asarray(bp2, np.float32).reshape(1, 1),
    )
    in_maps = [
        dict(common, xptl=xptb[G * r : G * (r + 1)], idx16=idx16[r], ewt=ewt[r])
        for r in range(R)
    ]

    res = run_bass_kernel_spmd(nc, in_maps, core_ids=list(range(R)), trace=False)

    y = np.zeros(N, np.float32)
    for r in range(R):
        yflat = res.results[r]["y"].T.reshape(-1)
        y[perm[r]] = yflat[DUM : DUM + perm[r].shape[0]]
    if DEBUG_TAPS:
        return y, res, (perm, gidx, ewt, idx16, meta)
    return y



# revision 6
# speedup vs baseline: 1.5346x; 1.5346x over previous
"""Trainium2 Bass kernel for nn_GATPredictor (2-layer multi-head GAT + MLP).

kernel(**inputs) takes FULL unsharded numpy inputs, returns the FULL
(50000,) float32 output.  8-way dst-node sharding.  Edges-on-partitions
aggregation: per dst-group, edges are packed densely into 128-edge
blocks; a static binary one-hot matrix M (edge x dst) scatters the
softmax-weighted messages into PSUM via the PE array.  Tables are bf16;
the layer-1 table is AllGathered in 7 chunks overlapped with layer-0
compute.
"""

import numpy as np
import ml_dtypes

BF16 = ml_dtypes.bfloat16

N = 50000
E = 800000
F_IN = 64
H = 4
C = 128
NEG = 0.2
R = 8
PER = N // R  # 6250
G = 49
CHUNK = G * 128  # 6272
GPC = 7  # groups per AllGather chunk
NCH = 7  # chunks
CROWS = R * GPC * 128  # 7168 rows per chunk
NPOS = R * CHUNK  # 50176
HALFA = 4 * CROWS  # 28672 rows in half A (chunks 0-3)
HALFB = NPOS - HALFA  # 21504
CJ = 16  # max blocks per gather call
ROW0 = 256  # bf16: [h 128 | asrc 4 | pad]   -> 512B
ROW1 = 640  # bf16: [x1 512 | asrc 4 | pad]  -> 1280B


# ---------------------------------------------------------------- host prep
def _prep_graph(edge_index, edge_weight):
    src = np.concatenate([np.asarray(edge_index[0], np.int64), np.arange(N)])
    dst = np.concatenate([np.asarray(edge_index[1], np.int64), np.arange(N)])
    ew = np.concatenate([np.asarray(edge_weight, np.float32), np.ones(N, np.float32)])

    deg = np.bincount(dst, minlength=N)
    gorder = np.argsort(deg, kind="stable")
    rank_of_node = np.empty(N, np.int64)
    rank_of_node[gorder] = np.arange(N) % R

    pos_row = np.empty(N, np.int64)
    gq = np.empty(N, np.int64)
    pq = np.empty(N, np.int64)
    perm_per_rank = []
    for r in range(R):
        nodes = np.where(rank_of_node == r)[0]
        order = np.argsort(deg[nodes], kind="stable")
        sn = nodes[order]
        perm_per_rank.append(sn)
        k = np.arange(sn.shape[0])
        g = k // 128
        p = k % 128
        pos_row[sn] = CROWS * (g // GPC) + (GPC * 128) * r + 128 * (g % GPC) + p
        gq[sn] = g
        pq[sn] = p

    src_row = pos_row[src]
    e_half = (src_row >= HALFA).astype(np.int64)
    e_inhalf = np.where(e_half == 0, src_row, src_row - HALFA)
    dst_r = rank_of_node[dst]
    dst_g = gq[dst]
    dst_p = pq[dst]

    counts = np.zeros((R, G, 2), np.int64)
    np.add.at(counts, (dst_r, dst_g, e_half), 1)
    cblk = np.ceil(counts / 128).astype(np.int64).max(axis=0)  # (G, 2)

    blockbase = np.zeros((G, 2), np.int64)
    acc = 0
    for g in range(G):
        for h in range(2):
            blockbase[g, h] = acc
            acc += int(cblk[g, h])
    NBLK = acc

    call_plan = []  # (g, h, b0, c)
    for g in range(G):
        for h in range(2):
            b = int(blockbase[g, h])
            left = int(cblk[g, h])
            while left > 0:
                c = min(CJ, left)
                call_plan.append((g, h, b, c))
                b += c
                left -= c

    # slot positions within each (r, g, h) run
    key = (dst_r * G + dst_g) * 2 + e_half
    order = np.argsort(key, kind="stable")
    sk = key[order]
    change = np.empty(sk.shape[0], np.bool_)
    change[0] = True
    change[1:] = sk[1:] != sk[:-1]
    run_start = np.maximum.accumulate(
        np.where(change, np.arange(sk.shape[0]), 0)
    )
    slot = np.arange(sk.shape[0]) - run_start

    er = dst_r[order]
    eg = dst_g[order]
    eh = e_half[order]
    ep = dst_p[order]
    ei = e_inhalf[order]
    eww = ew[order]
    eblk = blockbase[eg, eh] + slot // 128
    epp = slot % 128  # partition (edge lane)

    gidx = np.zeros((R, NBLK, 128), np.int16)
    Mv = np.zeros((R, NBLK, 128, 128), BF16)
    ewk = np.zeros((R, 128, NBLK), np.float32)
    gidx[er, eblk, epp] = ei.astype(np.int16)
    Mv[er, eblk, epp, ep] = 1.0
    ewk[er, epp, eblk] = eww
    Mp = np.ascontiguousarray(Mv.transpose(0, 2, 1, 3)).reshape(R, 128, NBLK * 128)
    Mtp = np.ascontiguousarray(Mv.transpose(0, 3, 1, 2)).reshape(R, 128, NBLK * 128)

    # idx16 packing: per call, linear order i = j*128 + p, wrapped in 16
    # partitions and replicated 8x down the partition dim
    idx16 = np.zeros((R, 128, 8 * NBLK), np.int16)
    for g, h, b0, c in call_plan:
        stream = gidx[:, b0 : b0 + c, :].reshape(R, -1)  # (R, c*128) j-major
        w = stream.reshape(R, c * 8, 16).transpose(0, 2, 1)  # (R, 16, 8c)
        idx16[:, :, 8 * b0 : 8 * (b0 + c)] = np.tile(w, (1, 8, 1))

    meta = dict(call_plan=tuple(call_plan), nblk=NBLK,
                cblk=tuple(int(cblk[g, 0]) + int(cblk[g, 1]) for g in range(G)))
    return perm_per_rank, idx16, Mp, Mtp, ewk.astype(BF16), meta


def _prep_weights(W_in, b_in, lin0, a_src0, a_dst0, lin1, a_src1, a_dst1):
    def fold(lin, a):
        return np.einsum(
            "ihc,hc->ih",
            lin.reshape(lin.shape[0], H, C).astype(np.float64),
            a.astype(np.float64),
        ).astype(np.float32)

    w_src0, w_dst0 = fold(lin0, a_src0), fold(lin0, a_dst0)
    rhs0 = np.zeros((128, 136), np.float32)
    rhs0[:F_IN, :C] = W_in
    rhs0[:F_IN, C : C + 4] = W_in @ w_src0
    rhs0[:F_IN, C + 4 :] = W_in @ w_dst0
    rhs0[F_IN, :C] = b_in
    rhs0[F_IN, C : C + 4] = b_in @ w_src0
    rhs0[F_IN, C + 4 :] = b_in @ w_dst0
    rhs0L = np.zeros((128, 132), np.float32)
    rhs0L[:F_IN, :C] = W_in
    rhs0L[:F_IN, C:] = W_in @ w_dst0
    rhs0L[F_IN, :C] = b_in
    rhs0L[F_IN, C:] = b_in @ w_dst0
    wa1 = np.concatenate([fold(lin1, a_src1), fold(lin1, a_dst1)], axis=1)
    return rhs0.astype(BF16), rhs0L.astype(BF16), wa1.astype(np.float32)


def _build_xpt_blocks(X, perm_per_rank):
    # xptb[b] for b = r*G + g : [128 feats+bias, 128 nodes] transposed block
    xpt = np.zeros((128, R * CHUNK), np.float32)
    xpt[F_IN, :] = 1.0
    for r in range(R):
        cols = CHUNK * r + np.arange(perm_per_rank[r].shape[0])
        xpt[:F_IN, cols] = X[perm_per_rank[r]].T
    return xpt.astype(BF16)


# ---------------------------------------------------------------- bass build
def _build_nc(call_plan, cblk, NBLK):
    import concourse.bacc as bacc
    import concourse.bass_isa as bass_isa
    import concourse.mybir as mybir
    import concourse.tile as tile
    from concourse.masks import make_identity

    f32 = mybir.dt.float32
    bf16 = mybir.dt.bfloat16
    i16 = mybir.dt.int16
    AF = mybir.ActivationFunctionType
    OP = mybir.AluOpType
    AX = mybir.AxisListType

    nc = bacc.Bacc(
        "TRN2",
        target_bir_lowering=False,
        debug=False,
        enable_asserts=False,
        num_devices=R,
        num_swdge_queues=4,
    )

    xptb = nc.dram_tensor("xptb", [128, R * G * 128], bf16, kind="ExternalInput")
    xptl = nc.dram_tensor("xptl", [128, G * 128], bf16, kind="ExternalInput")
    rhs0_d = nc.dram_tensor("rhs0", [128, 136], bf16, kind="ExternalInput")
    rhs0l_d = nc.dram_tensor("rhs0l", [128, 132], bf16, kind="ExternalInput")
    lin0_d = nc.dram_tensor("lin0", [128, 512], f32, kind="ExternalInput")
    lin1_d = nc.dram_tensor("lin1", [512, 512], f32, kind="ExternalInput")
    wa1_d = nc.dram_tensor("wa1", [512, 8], f32, kind="ExternalInput")
    wp1_d = nc.dram_tensor("wp1", [640, 128], f32, kind="ExternalInput")
    brow_d = nc.dram_tensor("brow", [1, 1280], f32, kind="ExternalInput")
    bp2_d = nc.dram_tensor("bp2", [1, 1], f32, kind="ExternalInput")
    idx_d = nc.dram_tensor("idx16", [128, 8 * NBLK], i16, kind="ExternalInput")
    ewk_d = nc.dram_tensor("ewk", [128, NBLK], bf16, kind="ExternalInput")
    m_d = nc.dram_tensor("md", [128, NBLK * 128], bf16, kind="ExternalInput")
    mt_d = nc.dram_tensor("mtd", [128, NBLK * 128], bf16, kind="ExternalInput")
    y_d = nc.dram_tensor("y", [128, G], f32, kind="ExternalOutput")

    # calls grouped by dst group
    plan_by_group = [[] for _ in range(G)]
    for g, h, b0, c in call_plan:
        plan_by_group[g].append((h, b0, c))

    with tile.TileContext(nc) as tc:
        with tc.tile_pool(name="dram", bufs=1, space="DRAM") as dram, tc.tile_pool(
            name="const", bufs=1
        ) as cp:
            table0 = dram.tile([NPOS, ROW0], bf16)
            table1 = dram.tile([NPOS, ROW1], bf16)
            agin = dram.tile([CHUNK, ROW1], bf16)
            h_res_d = dram.tile([G, 128, 128], f32)

            ident = cp.tile([128, 128], f32)
            make_identity(nc, ident[:])
            rhs0_s = cp.tile([128, 136], bf16)
            nc.sync.dma_start(out=rhs0_s[:], in_=rhs0_d[:, :])
            rhs0l_s = cp.tile([128, 132], bf16)
            nc.sync.dma_start(out=rhs0l_s[:], in_=rhs0l_d[:, :])
            lin0_s = cp.tile([128, 512], f32)
            nc.sync.dma_start(out=lin0_s[:], in_=lin0_d[:, :])
            lin1_s = [cp.tile([128, 512], f32, tag=f"lin1_{c_}", name=f"lin1s{c_}") for c_ in range(4)]
            wa1_s = [cp.tile([128, 8], f32, tag=f"wa1_{c_}", name=f"wa1s{c_}") for c_ in range(4)]
            for c_ in range(4):
                nc.sync.dma_start(out=lin1_s[c_][:], in_=lin1_d[128 * c_ : 128 * (c_ + 1), :])
                nc.sync.dma_start(out=wa1_s[c_][:], in_=wa1_d[128 * c_ : 128 * (c_ + 1), :])
            wp1_s = [cp.tile([128, 128], f32, tag=f"wp1_{c_}", name=f"wp1s{c_}") for c_ in range(5)]
            for c_ in range(5):
                nc.sync.dma_start(out=wp1_s[c_][:], in_=wp1_d[128 * c_ : 128 * (c_ + 1), :])
            brow = cp.tile([128, 1280], f32)
            nc.sync.dma_start(out=brow[0:1, :], in_=brow_d[:, :])
            nc.gpsimd.partition_broadcast(brow[:], brow[0:1, :])
            bias0r = brow[:, 0:512]
            bias1r = brow[:, 512:1024]
            bp1r = brow[:, 1024:1152]
            wp2r = brow[:, 1152:1280]
            bp2t = cp.tile([128, 1], f32)
            nc.sync.dma_start(out=bp2t[0:1, :], in_=bp2_d[:, :])
            nc.gpsimd.partition_broadcast(bp2t[:], bp2t[0:1, :])
            idx_s = cp.tile([128, 8 * NBLK], i16)
            nc.sync.dma_start(out=idx_s[:], in_=idx_d[:, :])
            ewk_s = cp.tile([128, NBLK], bf16)
            nc.sync.dma_start(out=ewk_s[:], in_=ewk_d[:, :])
            adst0_s = cp.tile([128, G * 4], bf16)
            adst1_s = cp.tile([128, G * 4], bf16)
            ysb = cp.tile([128, G], f32)
            m0t = cp.tile([128, 4], f32)
            m1t = cp.tile([128, 4], f32)
            m0tb = cp.tile([128, 4], bf16)
            m1tb = cp.tile([128, 4], bf16)
            mtmp = cp.tile([128, 4], f32)
            amax = cp.tile([128, 16], f32)
            nc.vector.memset(amax[:], -1e30)

            # ---------------- M0L: local h_res + adst0 (runs first)
            with nc.named_scope("m0l"), tc.tile_pool(name="m0ls", bufs=3) as lp, tc.tile_pool(
                name="m0lp", bufs=2, space="PSUM"
            ) as lpp:
                for ch in range(NCH):
                    xl7 = lp.tile([128, GPC, 128], bf16, tag="xl7")
                    nc.sync.dma_start(
                        out=xl7[:],
                        in_=xptl[:, GPC * 128 * ch : GPC * 128 * (ch + 1)],
                    )
                    for gj in range(GPC):
                        g = GPC * ch + gj
                        ps = lpp.tile([128, 132], f32, tag="m0lps")
                        nc.tensor.matmul(
                            ps[:], lhsT=xl7[:, gj, :], rhs=rhs0l_s[:], start=True, stop=True
                        )
                        stgL = lp.tile([128, 128], f32, tag="stgL")
                        nc.scalar.copy(out=stgL[:], in_=ps[:, 0:128])
                        nc.sync.dma_start(out=h_res_d[g, :, :], in_=stgL[:])
                        nc.vector.tensor_copy(
                            out=adst0_s[:, 4 * g : 4 * (g + 1)], in_=ps[:, 128:132]
                        )
                        nc.vector.tensor_tensor(
                            out=amax[:, 4:8], in0=amax[:, 4:8], in1=ps[:, 128:132], op=OP.max
                        )

            # ---------------- M0: replicated table0 (chunk-major so half A
            # completes early and layer-0 gathers can start)
            with nc.named_scope("m0"), tc.tile_pool(name="m0s", bufs=3) as mp, tc.tile_pool(
                name="m0p", bufs=2, space="PSUM"
            ) as mpp:
                for ch in range(NCH):
                    for r in range(R):
                        b0 = r * G + ch * GPC
                        xb7 = mp.tile([128, GPC, 128], bf16, tag="xb7")
                        nc.sync.dma_start(
                            out=xb7[:],
                            in_=xptb[:, 128 * b0 : 128 * (b0 + GPC)],
                        )
                        stg7 = mp.tile([128, GPC, ROW0], bf16, tag="stg7")
                        for gj in range(GPC):
                            ps = mpp.tile([128, 136], f32, tag="m0ps")
                            nc.tensor.matmul(
                                ps[:], lhsT=xb7[:, gj, :], rhs=rhs0_s[:], start=True, stop=True
                            )
                            nc.scalar.copy(out=stg7[:, gj, 0:132], in_=ps[:, 0:132])
                            nc.vector.tensor_tensor(
                                out=amax[:, 0:4], in0=amax[:, 0:4], in1=ps[:, 128:132],
                                op=OP.max,
                            )
                        base = CROWS * ch + (GPC * 128) * r
                        nc.sync.dma_start(
                            out=table0[base : base + GPC * 128, :].rearrange(
                                "(j p) e -> p j e", j=GPC
                            ),
                            in_=stg7[:],
                        )

            nc.gpsimd.partition_all_reduce(
                amax[:, 0:8], amax[:, 0:8], 128, bass_isa.ReduceOp.max
            )
            nc.vector.tensor_tensor(out=m0t[:], in0=amax[:, 0:4], in1=amax[:, 4:8], op=OP.add)
            nc.vector.tensor_scalar(
                out=mtmp[:], in0=m0t[:], scalar1=NEG, scalar2=None, op0=OP.mult
            )
            nc.vector.tensor_tensor(out=m0t[:], in0=m0t[:], in1=mtmp[:], op=OP.max)
            nc.vector.tensor_copy(out=m0tb[:], in_=m0t[:])

            qn = [0]

            def gather(out_ap, half_ap, b0, c, elem):
                nc.gpsimd.dma_gather(
                    out_ap=out_ap,
                    in_ap=half_ap,
                    idxs_ap=idx_s[:, 8 * b0 : 8 * (b0 + c)],
                    num_idxs=128 * c,
                    num_idxs_reg=128 * c,
                    elem_size=elem,
                    single_packet=False,
                    queue_num=qn[0] % 4,
                )
                qn[0] += 1

            # ---------------- shared aggregation loop body
            def agg_layer(g, table, row, adst_s, mtb, gp, mtp, ep, accp, saccp, edstp, wp):
                calls = plan_by_group[g]
                nblocks = cblk[g]
                accps = accp.tile([128, 512], f32, tag="acc")
                saccps = saccp.tile([128, 4], f32, tag="sacc")
                bi = 0
                for h, b0, c in calls:
                    xg = gp.tile([128, CJ, row], bf16, tag="xg")
                    gather(
                        xg[:, 0:c, :],
                        table[0:HALFA, :] if h == 0 else table[HALFA:NPOS, :],
                        b0, c, row,
                    )
                    mtile = mtp.tile([128, CJ, 128], bf16, tag="m")
                    nc.sync.dma_start(
                        out=mtile[:, 0:c, :],
                        in_=m_d[:, 128 * b0 : 128 * (b0 + c)],
                    )
                    mttile = mtp.tile([128, CJ, 128], bf16, tag="mt")
                    nc.sync.dma_start(
                        out=mttile[:, 0:c, :],
                        in_=mt_d[:, 128 * b0 : 128 * (b0 + c)],
                    )
                    edst_ps = edstp.tile([128, CJ, 4], f32, tag="edst")
                    for j in range(c):
                        nc.tensor.matmul(
                            edst_ps[:, j, :],
                            lhsT=mttile[:, j, :],
                            rhs=adst_s[:, 4 * g : 4 * (g + 1)],
                            start=True, stop=True,
                        )
                    edst_sb = ep.tile([128, CJ, 4], bf16, tag="edsb")
                    nc.scalar.copy(out=edst_sb[:, 0:c, :], in_=edst_ps[:, 0:c, :])
                    asoff = 128 if row == ROW0 else 512
                    et = ep.tile([128, CJ, 4], bf16, tag="et")
                    nc.vector.tensor_tensor(
                        out=et[:, 0:c, :], in0=xg[:, 0:c, asoff : asoff + 4],
                        in1=edst_sb[:, 0:c, :], op=OP.add,
                    )
                    lr = ep.tile([128, CJ, 4], bf16, tag="lr")
                    nc.vector.tensor_scalar(
                        out=lr[:, 0:c, :], in0=et[:, 0:c, :], scalar1=NEG,
                        scalar2=None, op0=OP.mult,
                    )
                    nc.vector.tensor_tensor(
                        out=et[:, 0:c, :], in0=et[:, 0:c, :], in1=lr[:, 0:c, :], op=OP.max
                    )
                    nc.vector.tensor_tensor(
                        out=et[:, 0:c, :], in0=et[:, 0:c, :],
                        in1=mtb[:, None, :].to_broadcast([128, c, 4]), op=OP.subtract,
                    )
                    nc.scalar.activation(et[:, 0:c, :], et[:, 0:c, :], AF.Exp)
                    wt = ep.tile([128, CJ, 4], f32, tag="wt")
                    nc.vector.tensor_tensor(
                        out=wt[:, 0:c, :], in0=et[:, 0:c, :],
                        in1=ewk_s[:, b0 : b0 + c, None].to_broadcast([128, c, 4]),
                        op=OP.mult,
                    )
                    for j in range(c):
                        wx = wp.tile([128, 512], bf16, tag="wx")
                        for h4 in range(4):
                            nc.vector.tensor_scalar(
                                out=wx[:, 128 * h4 : 128 * (h4 + 1)],
                                in0=xg[:, j, 0:128] if row == ROW0
                                else xg[:, j, 128 * h4 : 128 * (h4 + 1)],
                                scalar1=wt[:, j : j + 1, h4],
                                scalar2=None, op0=OP.mult,
                            )
                        nc.tensor.matmul(
                            accps[:], lhsT=mtile[:, j, :], rhs=wx[:],
                            start=(bi == 0), stop=(bi == nblocks - 1),
                            skip_group_check=True,
                        )
                        nc.tensor.matmul(
                            saccps[:], lhsT=mtile[:, j, :], rhs=et[:, j, :],
                            start=(bi == 0), stop=(bi == nblocks - 1),
                            skip_group_check=True,
                        )
                        bi += 1
                return accps, saccps

            # ---------------- L0 + chunked AllGather
            with nc.named_scope("l0"), tc.tile_pool(name="l0g", bufs=3) as gp0, tc.tile_pool(
                name="l0m", bufs=2
            ) as mtp0, tc.tile_pool(name="l0e", bufs=4) as ep0, tc.tile_pool(
                name="l0w", bufs=6
            ) as wp0, tc.tile_pool(name="l0sp", bufs=2) as sp0, tc.tile_pool(
                name="l0acc", bufs=1, space="PSUM"
            ) as accp0, tc.tile_pool(
                name="l0sa", bufs=1, space="PSUM"
            ) as saccp0, tc.tile_pool(
                name="l0ed", bufs=1, space="PSUM"
            ) as edstp0, tc.tile_pool(
                name="l0out", bufs=1, space="PSUM"
            ) as outp0, tc.tile_pool(
                name="l0a", bufs=1, space="PSUM"
            ) as ap0, tc.tile_pool(
                name="l0t", bufs=2, space="PSUM"
            ) as trp0:
                for g in range(G):
                    accps, saccps = agg_layer(
                        g, table0, ROW0, adst0_s, m0tb,
                        gp0, mtp0, ep0, accp0, saccp0, edstp0, wp0,
                    )
                    srec = sp0.tile([128, 4], f32, tag="srec")
                    nc.vector.tensor_scalar(
                        out=srec[:], in0=saccps[:], scalar1=1e-16, scalar2=None, op0=OP.add
                    )
                    nc.vector.reciprocal(srec[:], srec[:])
                    acc_sb = sp0.tile([128, 512], f32, tag="accsb")
                    nc.scalar.copy(out=acc_sb[:], in_=accps[:])
                    aggT = sp0.tile([128, 512], f32, tag="aggT")
                    for hh in range(4):
                        trp = trp0.tile([128, 128], f32, tag="tr", name=f"tr0_{g}_{hh}")
                        nc.tensor.transpose(
                            out=trp[:], in_=acc_sb[:, 128 * hh : 128 * (hh + 1)],
                            identity=ident[:],
                        )
                        nc.scalar.copy(out=aggT[:, 128 * hh : 128 * (hh + 1)], in_=trp[:])
                    outps = outp0.tile([128, 512], f32, tag="out0")
                    for hh in range(4):
                        nc.tensor.matmul(
                            outps[:, 128 * hh : 128 * (hh + 1)],
                            lhsT=aggT[:, 128 * hh : 128 * (hh + 1)],
                            rhs=lin0_s[:, 128 * hh : 128 * (hh + 1)],
                            start=True, stop=True,
                        )
                    h1 = sp0.tile([128, 512], f32, tag="h1")
                    for hh in range(4):
                        nc.scalar.activation(
                            h1[:, 128 * hh : 128 * (hh + 1)],
                            outps[:, 128 * hh : 128 * (hh + 1)],
                            AF.Copy, scale=srec[:, hh : hh + 1],
                        )
                    nc.vector.tensor_tensor(out=h1[:], in0=h1[:], in1=bias0r, op=OP.add)
                    u = sp0.tile([128, 512], f32, tag="elu_u")
                    nc.vector.tensor_scalar(
                        out=u[:], in0=h1[:], scalar1=0.0, scalar2=None, op0=OP.min
                    )
                    v = sp0.tile([128, 512], f32, tag="elu_v")
                    nc.scalar.activation(v[:], u[:], AF.Exp)
                    nc.vector.tensor_tensor(out=h1[:], in0=h1[:], in1=u[:], op=OP.subtract)
                    nc.vector.tensor_tensor(out=h1[:], in0=h1[:], in1=v[:], op=OP.add)
                    nc.vector.tensor_scalar(
                        out=h1[:], in0=h1[:], scalar1=1.0, scalar2=None, op0=OP.subtract
                    )
                    h1T = sp0.tile([128, 512], f32, tag="h1T")
                    for cc in range(4):
                        trp = trp0.tile([128, 128], f32, tag="tr", name=f"trh1_{g}_{cc}")
                        nc.tensor.transpose(
                            out=trp[:], in_=h1[:, 128 * cc : 128 * (cc + 1)],
                            identity=ident[:],
                        )
                        nc.scalar.copy(out=h1T[:, 128 * cc : 128 * (cc + 1)], in_=trp[:])
                    x1ps = outp0.tile([128, 512], f32, tag="x1")
                    aps = ap0.tile([128, 8], f32, tag="aps")
                    for cc in range(4):
                        nc.tensor.matmul(
                            x1ps[:], lhsT=h1T[:, 128 * cc : 128 * (cc + 1)],
                            rhs=lin1_s[cc][:], start=(cc == 0), stop=(cc == 3),
                        )
                    for cc in range(4):
                        nc.tensor.matmul(
                            aps[:], lhsT=h1T[:, 128 * cc : 128 * (cc + 1)],
                            rhs=wa1_s[cc][:], start=(cc == 0), stop=(cc == 3),
                        )
                    stg = sp0.tile([128, ROW1], bf16, tag="stg1")
                    nc.scalar.copy(out=stg[:, 0:512], in_=x1ps[:])
                    nc.vector.tensor_copy(out=stg[:, 512:516], in_=aps[:, 0:4])
                    nc.vector.tensor_copy(
                        out=adst1_s[:, 4 * g : 4 * (g + 1)], in_=aps[:, 4:8]
                    )
                    nc.vector.tensor_tensor(
                        out=amax[:, 8:12], in0=amax[:, 8:12], in1=aps[:, 0:4], op=OP.max
                    )
                    nc.vector.tensor_tensor(
                        out=amax[:, 12:16], in0=amax[:, 12:16], in1=aps[:, 4:8], op=OP.max
                    )
                    nc.sync.dma_start(out=agin[128 * g : 128 * (g + 1), :], in_=stg[:])
                    if g % GPC == GPC - 1:
                        ch = g // GPC
                        with nc.named_scope(f"ag{ch}"):
                            nc.gpsimd.collective_compute(
                                "AllGather",
                                mybir.AluOpType.bypass,
                                replica_groups=[list(range(R))],
                                ins=[agin[GPC * 128 * ch : GPC * 128 * (ch + 1), :].opt()],
                                outs=[table1[CROWS * ch : CROWS * (ch + 1), :].opt()],
                            )

            nc.gpsimd.partition_all_reduce(
                amax[:, 8:16], amax[:, 8:16], 128, bass_isa.ReduceOp.max
            )
            nc.vector.tensor_tensor(
                out=m1t[:], in0=amax[:, 8:12], in1=amax[:, 12:16], op=OP.add
            )
            nc.vector.tensor_scalar(
                out=mtmp[:], in0=m1t[:], scalar1=NEG, scalar2=None, op0=OP.mult
            )
            nc.vector.tensor_tensor(out=m1t[:], in0=m1t[:], in1=mtmp[:], op=OP.max)
            nc.vector.tensor_copy(out=m1tb[:], in_=m1t[:])

            # ---------------- L1 + final MLP
            with nc.named_scope("l1"), tc.tile_pool(name="l1g", bufs=3) as gp1, tc.tile_pool(
                name="l1m", bufs=2
            ) as mtp1, tc.tile_pool(name="l1e", bufs=4) as ep1, tc.tile_pool(
                name="l1w", bufs=6
            ) as wp1p, tc.tile_pool(name="l1sp", bufs=2) as sp1, tc.tile_pool(
                name="l1acc", bufs=2, space="PSUM"
            ) as accp1, tc.tile_pool(
                name="l1sa", bufs=1, space="PSUM"
            ) as saccp1, tc.tile_pool(
                name="l1ed", bufs=1, space="PSUM"
            ) as edstp1, tc.tile_pool(
                name="l1z", bufs=1, space="PSUM"
            ) as zp1, tc.tile_pool(
                name="l1t", bufs=2, space="PSUM"
            ) as trp1:
                for g in range(G):
                    accps, saccps = agg_layer(
                        g, table1, ROW1, adst1_s, m1tb,
                        gp1, mtp1, ep1, accp1, saccp1, edstp1, wp1p,
                    )
                    srec = sp1.tile([128, 4], f32, tag="srec")
                    nc.vector.tensor_scalar(
                        out=srec[:], in0=saccps[:], scalar1=1e-16, scalar2=None, op0=OP.add
                    )
                    nc.vector.reciprocal(srec[:], srec[:])
                    h2 = sp1.tile([128, 512], f32, tag="h2")
                    for hh in range(4):
                        nc.scalar.activation(
                            h2[:, 128 * hh : 128 * (hh + 1)],
                            accps[:, 128 * hh : 128 * (hh + 1)],
                            AF.Copy, scale=srec[:, hh : hh + 1],
                        )
                    nc.vector.tensor_tensor(out=h2[:], in0=h2[:], in1=bias1r, op=OP.add)
                    u = sp1.tile([128, 512], f32, tag="elu_u")
                    nc.vector.tensor_scalar(
                        out=u[:], in0=h2[:], scalar1=0.0, scalar2=None, op0=OP.min
                    )
                    v = sp1.tile([128, 512], f32, tag="elu_v")
                    nc.scalar.activation(v[:], u[:], AF.Exp)
                    nc.vector.tensor_tensor(out=h2[:], in0=h2[:], in1=u[:], op=OP.subtract)
                    nc.vector.tensor_tensor(out=h2[:], in0=h2[:], in1=v[:], op=OP.add)
                    nc.vector.tensor_scalar(
                        out=h2[:], in0=h2[:], scalar1=1.0, scalar2=None, op0=OP.subtract
                    )
                    hT = sp1.tile([128, 640], f32, tag="hT")
                    for cc in range(4):
                        trp = trp1.tile([128, 128], f32, tag="tr", name=f"trh2_{g}_{cc}")
                        nc.tensor.transpose(
                            out=trp[:], in_=h2[:, 128 * cc : 128 * (cc + 1)],
                            identity=ident[:],
                        )
                        nc.scalar.copy(out=hT[:, 128 * cc : 128 * (cc + 1)], in_=trp[:])
                    hrb = sp1.tile([128, 128], f32, tag="hrb")
                    nc.sync.dma_start(out=hrb[:], in_=h_res_d[g, :, :])
                    trp = trp1.tile([128, 128], f32, tag="tr", name=f"trhr_{g}")
                    nc.tensor.transpose(out=trp[:], in_=hrb[:], identity=ident[:])
                    nc.scalar.copy(out=hT[:, 512:640], in_=trp[:])
                    zps = zp1.tile([128, 128], f32, tag="z")
                    for cc in range(5):
                        nc.tensor.matmul(
                            zps[:], lhsT=hT[:, 128 * cc : 128 * (cc + 1)],
                            rhs=wp1_s[cc][:], start=(cc == 0), stop=(cc == 4),
                        )
                    zsb = sp1.tile([128, 128], f32, tag="zsb")
                    nc.scalar.copy(out=zsb[:], in_=zps[:])
                    nc.vector.tensor_tensor(out=zsb[:], in0=zsb[:], in1=bp1r, op=OP.add)
                    nc.scalar.activation(zsb[:], zsb[:], AF.Gelu_apprx_tanh)
                    nc.vector.tensor_tensor(out=zsb[:], in0=zsb[:], in1=wp2r, op=OP.mult)
                    yred = sp1.tile([128, 1], f32, tag="yred")
                    nc.vector.tensor_reduce(yred[:], zsb[:], axis=AX.X, op=OP.add)
                    nc.vector.tensor_scalar(
                        out=ysb[:, g : g + 1], in0=yred[:], scalar1=bp2t[:, 0:1],
                        scalar2=None, op0=OP.add,
                    )
            nc.sync.dma_start(out=y_d[:, :], in_=ysb[:])

    nc.compile()
    return nc


_CACHE = {}


def kernel(
    X,
    edge_index,
    edge_weight,
    W_in,
    b_in,
    lin0,
    att_src0,
    att_dst0,
    bias0,
    lin1,
    att_src1,
    att_dst1,
    bias1,
    Wp1,
    bp1,
    Wp2,
    bp2,
):
    from concourse.bass_utils import run_bass_kernel_spmd

    X = np.asarray(X, np.float32)
    perm, idx16, Mv, Mt, ewk, meta = _prep_graph(
        np.asarray(edge_index), np.asarray(edge_weight)
    )
    rhs0b, rhs0lb, wa1 = _prep_weights(
        np.asarray(W_in, np.float32),
        np.asarray(b_in, np.float32),
        np.asarray(lin0, np.float32),
        np.asarray(att_src0, np.float32),
        np.asarray(att_dst0, np.float32),
        np.asarray(lin1, np.float32),
        np.asarray(att_src1, np.float32),
        np.asarray(att_dst1, np.float32),
    )
    xptb = _build_xpt_blocks(X, perm)

    key = ("nc", meta["nblk"], meta["call_plan"], meta["cblk"])
    if key not in _CACHE:
        _CACHE[key] = _build_nc(list(meta["call_plan"]), list(meta["cblk"]), meta["nblk"])
    nc = _CACHE[key]

    brow = np.zeros((1, 1280), np.float32)
    brow[0, 0:512] = np.asarray(bias0, np.float32)
    brow[0, 512:1024] = np.asarray(bias1, np.float32)
    brow[0, 1024:1152] = np.asarray(bp1, np.float32)
    brow[0, 1152:1280] = np.asarray(Wp2, np.float32)[:, 0]

    common = dict(
        xptb=xptb,
        rhs0=rhs0b,
        rhs0l=rhs0lb,
        lin0=np.ascontiguousarray(np.asarray(lin0, np.float32)),
        lin1=np.ascontiguousarray(np.asarray(lin1, np.float32)),
        wa1=wa1,
        wp1=np.ascontiguousarray(np.asarray(Wp1, np.float32)),
        brow=brow,
        bp2=np.asarray(bp2, np.float32).reshape(1, 1),
    )
    in_maps = [
        dict(
            common,
            xptl=np.ascontiguousarray(xptb[:, G * 128 * r : G * 128 * (r + 1)]),
            idx16=idx16[r],
            ewk=ewk[r],
            md=Mv[r],
            mtd=Mt[r],
        )
        for r in range(R)
    ]

    res = run_bass_kernel_spmd(nc, in_maps, core_ids=list(range(R)), trace=False)

    y = np.zeros(N, np.float32)
    for r in range(R):
        yflat = res.results[r]["y"].T.reshape(-1)
        y[perm[r]] = yflat[: perm[r].shape[0]]
    return y


# revision 8
# speedup vs baseline: 1.7709x; 1.1540x over previous
"""Trainium2 Bass kernel for nn_GATPredictor (2-layer multi-head GAT + MLP).

kernel(**inputs) takes FULL unsharded numpy inputs, returns the FULL
(50000,) float32 output.  8-way dst-node sharding.  Edges-on-partitions
aggregation: per dst-group, edges are packed densely into 128-edge
blocks; a static binary one-hot matrix M (edge x dst) scatters the
softmax-weighted messages into PSUM via the PE array.  Tables are bf16;
the layer-1 table is AllGathered in 7 chunks overlapped with layer-0
compute.
"""

import numpy as np
import ml_dtypes

BF16 = ml_dtypes.bfloat16

N = 50000
E = 800000
F_IN = 64
H = 4
C = 128
NEG = 0.2
R = 8
PER = N // R  # 6250
G = 49
CHUNK = G * 128  # 6272
GPC = 7  # groups per AllGather chunk
NCH = 7  # chunks
CROWS = R * GPC * 128  # 7168 rows per chunk
NPOS = R * CHUNK  # 50176
HALFA = 4 * CROWS  # 28672 rows in half A (chunks 0-3)
HALFB = NPOS - HALFA  # 21504
CJ = 16  # max blocks per gather call
ROW0 = 256  # bf16: [h 128 | asrc 4 | pad]   -> 512B
ROW1 = 640  # bf16: [x1 512 | asrc 4 | pad]  -> 1280B


# ---------------------------------------------------------------- host prep
def _prep_graph(edge_index, edge_weight):
    src = np.concatenate([np.asarray(edge_index[0], np.int64), np.arange(N)])
    dst = np.concatenate([np.asarray(edge_index[1], np.int64), np.arange(N)])
    ew = np.concatenate([np.asarray(edge_weight, np.float32), np.ones(N, np.float32)])

    deg = np.bincount(dst, minlength=N)
    gorder = np.argsort(deg, kind="stable")
    rank_of_node = np.empty(N, np.int64)
    rank_of_node[gorder] = np.arange(N) % R

    pos_row = np.empty(N, np.int64)
    gq = np.empty(N, np.int64)
    pq = np.empty(N, np.int64)
    perm_per_rank = []
    for r in range(R):
        nodes = np.where(rank_of_node == r)[0]
        order = np.argsort(deg[nodes], kind="stable")
        sn = nodes[order]
        perm_per_rank.append(sn)
        k = np.arange(sn.shape[0])
        g = k // 128
        p = k % 128
        pos_row[sn] = CROWS * (g // GPC) + (GPC * 128) * r + 128 * (g % GPC) + p
        gq[sn] = g
        pq[sn] = p

    src_row = pos_row[src]
    e_half = (src_row >= HALFA).astype(np.int64)
    e_inhalf = np.where(e_half == 0, src_row, src_row - HALFA)
    dst_r = rank_of_node[dst]
    dst_g = gq[dst]
    dst_p = pq[dst]

    counts = np.zeros((R, G, 2), np.int64)
    np.add.at(counts, (dst_r, dst_g, e_half), 1)
    cblk = np.ceil(counts / 128).astype(np.int64).max(axis=0)  # (G, 2)

    blockbase = np.zeros((G, 2), np.int64)
    acc = 0
    for g in range(G):
        for h in range(2):
            blockbase[g, h] = acc
            acc += int(cblk[g, h])
    NBLK = acc

    call_plan = []  # (g, h, b0, c)
    for g in range(G):
        for h in range(2):
            b = int(blockbase[g, h])
            left = int(cblk[g, h])
            while left > 0:
                c = min(CJ, left)
                call_plan.append((g, h, b, c))
                b += c
                left -= c

    # slot positions within each (r, g, h) run
    key = (dst_r * G + dst_g) * 2 + e_half
    order = np.argsort(key, kind="stable")
    sk = key[order]
    change = np.empty(sk.shape[0], np.bool_)
    change[0] = True
    change[1:] = sk[1:] != sk[:-1]
    run_start = np.maximum.accumulate(
        np.where(change, np.arange(sk.shape[0]), 0)
    )
    slot = np.arange(sk.shape[0]) - run_start

    er = dst_r[order]
    eg = dst_g[order]
    eh = e_half[order]
    ep = dst_p[order]
    ei = e_inhalf[order]
    eww = ew[order]
    eblk = blockbase[eg, eh] + slot // 128
    epp = slot % 128  # partition (edge lane)

    gidx = np.zeros((R, NBLK, 128), np.int16)
    Mv = np.zeros((R, NBLK, 128, 128), BF16)
    ewk = np.zeros((R, 128, NBLK), np.float32)
    gidx[er, eblk, epp] = ei.astype(np.int16)
    Mv[er, eblk, epp, ep] = 1.0
    ewk[er, epp, eblk] = eww
    Mp = np.ascontiguousarray(Mv.transpose(0, 2, 1, 3)).reshape(R, 128, NBLK * 128)
    Mtp = np.ascontiguousarray(Mv.transpose(0, 3, 1, 2)).reshape(R, 128, NBLK * 128)

    # idx16 packing: per call, linear order i = j*128 + p, wrapped in 16
    # partitions and replicated 8x down the partition dim
    idx16 = np.zeros((R, 128, 8 * NBLK), np.int16)
    for g, h, b0, c in call_plan:
        stream = gidx[:, b0 : b0 + c, :].reshape(R, -1)  # (R, c*128) j-major
        w = stream.reshape(R, c * 8, 16).transpose(0, 2, 1)  # (R, 16, 8c)
        idx16[:, :, 8 * b0 : 8 * (b0 + c)] = np.tile(w, (1, 8, 1))

    meta = dict(call_plan=tuple(call_plan), nblk=NBLK,
                cblk=tuple(int(cblk[g, 0]) + int(cblk[g, 1]) for g in range(G)))
    return perm_per_rank, idx16, Mp, Mtp, ewk.astype(BF16), meta


def _prep_weights(W_in, b_in, lin0, a_src0, a_dst0, lin1, a_src1, a_dst1):
    def fold(lin, a):
        return np.einsum(
            "ihc,hc->ih",
            lin.reshape(lin.shape[0], H, C).astype(np.float64),
            a.astype(np.float64),
        ).astype(np.float32)

    w_src0, w_dst0 = fold(lin0, a_src0), fold(lin0, a_dst0)
    rhs0 = np.zeros((128, 136), np.float32)
    rhs0[:F_IN, :C] = W_in
    rhs0[:F_IN, C : C + 4] = W_in @ w_src0
    rhs0[:F_IN, C + 4 :] = W_in @ w_dst0
    rhs0[F_IN, :C] = b_in
    rhs0[F_IN, C : C + 4] = b_in @ w_src0
    rhs0[F_IN, C + 4 :] = b_in @ w_dst0
    rhs0L = np.zeros((128, 132), np.float32)
    rhs0L[:F_IN, :C] = W_in
    rhs0L[:F_IN, C:] = W_in @ w_dst0
    rhs0L[F_IN, :C] = b_in
    rhs0L[F_IN, C:] = b_in @ w_dst0
    wa1 = np.concatenate([fold(lin1, a_src1), fold(lin1, a_dst1)], axis=1)
    return rhs0.astype(BF16), rhs0L.astype(BF16), wa1.astype(np.float32)


def _build_xpt_blocks(X, perm_per_rank):
    # xptb[b] for b = r*G + g : [128 feats+bias, 128 nodes] transposed block
    xpt = np.zeros((128, R * CHUNK), np.float32)
    xpt[F_IN, :] = 1.0
    for r in range(R):
        cols = CHUNK * r + np.arange(perm_per_rank[r].shape[0])
        xpt[:F_IN, cols] = X[perm_per_rank[r]].T
    return xpt.astype(BF16)


# ---------------------------------------------------------------- bass build
def _build_nc(call_plan, cblk, NBLK):
    import concourse.bacc as bacc
    import concourse.bass_isa as bass_isa
    import concourse.mybir as mybir
    import concourse.tile as tile
    from concourse.masks import make_identity

    f32 = mybir.dt.float32
    bf16 = mybir.dt.bfloat16
    i16 = mybir.dt.int16
    AF = mybir.ActivationFunctionType
    OP = mybir.AluOpType
    AX = mybir.AxisListType

    nc = bacc.Bacc(
        "TRN2",
        target_bir_lowering=False,
        debug=False,
        enable_asserts=False,
        num_devices=R,
        num_swdge_queues=4,
    )

    xptb = nc.dram_tensor("xptb", [128, R * G * 128], bf16, kind="ExternalInput")
    xptl = nc.dram_tensor("xptl", [128, G * 128], bf16, kind="ExternalInput")
    rhs0_d = nc.dram_tensor("rhs0", [128, 136], bf16, kind="ExternalInput")
    rhs0l_d = nc.dram_tensor("rhs0l", [128, 132], bf16, kind="ExternalInput")
    lin0_d = nc.dram_tensor("lin0", [128, 512], f32, kind="ExternalInput")
    lin1_d = nc.dram_tensor("lin1", [512, 512], f32, kind="ExternalInput")
    wa1_d = nc.dram_tensor("wa1", [512, 8], f32, kind="ExternalInput")
    wp1_d = nc.dram_tensor("wp1", [640, 128], f32, kind="ExternalInput")
    brow_d = nc.dram_tensor("brow", [1, 1280], f32, kind="ExternalInput")
    bp2_d = nc.dram_tensor("bp2", [1, 1], f32, kind="ExternalInput")
    idx_d = nc.dram_tensor("idx16", [128, 8 * NBLK], i16, kind="ExternalInput")
    ewk_d = nc.dram_tensor("ewk", [128, NBLK], bf16, kind="ExternalInput")
    m_d = nc.dram_tensor("md", [128, NBLK * 128], bf16, kind="ExternalInput")
    mt_d = nc.dram_tensor("mtd", [128, NBLK * 128], bf16, kind="ExternalInput")
    y_d = nc.dram_tensor("y", [128, G], f32, kind="ExternalOutput")

    # calls grouped by dst group
    plan_by_group = [[] for _ in range(G)]
    for g, h, b0, c in call_plan:
        plan_by_group[g].append((h, b0, c))

    with tile.TileContext(nc) as tc:
        with tc.tile_pool(name="dram", bufs=1, space="DRAM") as dram, tc.tile_pool(
            name="const", bufs=1
        ) as cp:
            table0 = dram.tile([NPOS, ROW0], bf16)
            table1 = dram.tile([NPOS, ROW1], bf16)
            agin = dram.tile([CHUNK, ROW1], bf16)
            h_res_d = dram.tile([G, 128, 128], f32)

            ident = cp.tile([128, 128], f32)
            make_identity(nc, ident[:])
            rhs0_s = cp.tile([128, 136], bf16)
            nc.sync.dma_start(out=rhs0_s[:], in_=rhs0_d[:, :])
            rhs0l_s = cp.tile([128, 132], bf16)
            nc.sync.dma_start(out=rhs0l_s[:], in_=rhs0l_d[:, :])
            lin0_s = cp.tile([128, 512], f32)
            nc.sync.dma_start(out=lin0_s[:], in_=lin0_d[:, :])
            lin1_s = [cp.tile([128, 512], f32, tag=f"lin1_{c_}", name=f"lin1s{c_}") for c_ in range(4)]
            wa1_s = [cp.tile([128, 8], f32, tag=f"wa1_{c_}", name=f"wa1s{c_}") for c_ in range(4)]
            for c_ in range(4):
                nc.sync.dma_start(out=lin1_s[c_][:], in_=lin1_d[128 * c_ : 128 * (c_ + 1), :])
                nc.sync.dma_start(out=wa1_s[c_][:], in_=wa1_d[128 * c_ : 128 * (c_ + 1), :])
            wp1_s = [cp.tile([128, 128], f32, tag=f"wp1_{c_}", name=f"wp1s{c_}") for c_ in range(5)]
            for c_ in range(5):
                nc.sync.dma_start(out=wp1_s[c_][:], in_=wp1_d[128 * c_ : 128 * (c_ + 1), :])
            brow = cp.tile([128, 1280], f32)
            nc.sync.dma_start(out=brow[0:1, :], in_=brow_d[:, :])
            nc.gpsimd.partition_broadcast(brow[:], brow[0:1, :])
            bias0r = brow[:, 0:512]
            bias1r = brow[:, 512:1024]
            bp1r = brow[:, 1024:1152]
            wp2r = brow[:, 1152:1280]
            bp2t = cp.tile([128, 1], f32)
            nc.sync.dma_start(out=bp2t[0:1, :], in_=bp2_d[:, :])
            nc.gpsimd.partition_broadcast(bp2t[:], bp2t[0:1, :])
            idx_s = cp.tile([128, 8 * NBLK], i16)
            nc.sync.dma_start(out=idx_s[:], in_=idx_d[:, :])
            ewk_s = cp.tile([128, NBLK], bf16)
            nc.sync.dma_start(out=ewk_s[:], in_=ewk_d[:, :])
            adst0_s = cp.tile([128, G * 4], bf16)
            adst1_s = cp.tile([128, G * 4], bf16)
            ysb = cp.tile([128, G], f32)
            m0t = cp.tile([128, 4], f32)
            m1t = cp.tile([128, 4], f32)
            m0tb = cp.tile([128, 4], bf16)
            m1tb = cp.tile([128, 4], bf16)
            mtmp = cp.tile([128, 4], f32)
            amax = cp.tile([128, 16], f32)
            nc.vector.memset(amax[:], -1e30)

            # ---------------- M0L: local h_res + adst0 (runs first)
            with nc.named_scope("m0l"), tc.tile_pool(name="m0ls", bufs=3) as lp, tc.tile_pool(
                name="m0lp", bufs=2, space="PSUM"
            ) as lpp:
                for ch in range(NCH):
                    xl7 = lp.tile([128, GPC, 128], bf16, tag="xl7")
                    nc.sync.dma_start(
                        out=xl7[:],
                        in_=xptl[:, GPC * 128 * ch : GPC * 128 * (ch + 1)],
                    )
                    for gj in range(GPC):
                        g = GPC * ch + gj
                        ps = lpp.tile([128, 132], f32, tag="m0lps")
                        nc.tensor.matmul(
                            ps[:], lhsT=xl7[:, gj, :], rhs=rhs0l_s[:], start=True, stop=True
                        )
                        stgL = lp.tile([128, 128], f32, tag="stgL")
                        nc.scalar.copy(out=stgL[:], in_=ps[:, 0:128])
                        nc.sync.dma_start(out=h_res_d[g, :, :], in_=stgL[:])
                        nc.vector.tensor_copy(
                            out=adst0_s[:, 4 * g : 4 * (g + 1)], in_=ps[:, 128:132]
                        )
                        nc.vector.tensor_tensor(
                            out=amax[:, 4:8], in0=amax[:, 4:8], in1=ps[:, 128:132], op=OP.max
                        )

            # ---------------- M0: replicated table0 (chunk-major so half A
            # completes early and layer-0 gathers can start)
            with nc.named_scope("m0"), tc.tile_pool(name="m0s", bufs=3) as mp, tc.tile_pool(
                name="m0p", bufs=2, space="PSUM"
            ) as mpp:
                for ch in range(NCH):
                    for r in range(R):
                        b0 = r * G + ch * GPC
                        xb7 = mp.tile([128, GPC, 128], bf16, tag="xb7")
                        nc.sync.dma_start(
                            out=xb7[:],
                            in_=xptb[:, 128 * b0 : 128 * (b0 + GPC)],
                        )
                        stg7 = mp.tile([128, GPC, ROW0], bf16, tag="stg7")
                        for gj in range(GPC):
                            ps = mpp.tile([128, 136], f32, tag="m0ps")
                            nc.tensor.matmul(
                                ps[:], lhsT=xb7[:, gj, :], rhs=rhs0_s[:], start=True, stop=True
                            )
                            nc.scalar.copy(out=stg7[:, gj, 0:132], in_=ps[:, 0:132])
                            nc.vector.tensor_tensor(
                                out=amax[:, 0:4], in0=amax[:, 0:4], in1=ps[:, 128:132],
                                op=OP.max,
                            )
                        base = CROWS * ch + (GPC * 128) * r
                        nc.sync.dma_start(
                            out=table0[base : base + GPC * 128, :].rearrange(
                                "(j p) e -> p j e", j=GPC
                            ),
                            in_=stg7[:],
                        )

            nc.gpsimd.partition_all_reduce(
                amax[:, 0:8], amax[:, 0:8], 128, bass_isa.ReduceOp.max
            )
            nc.vector.tensor_tensor(out=m0t[:], in0=amax[:, 0:4], in1=amax[:, 4:8], op=OP.add)
            nc.vector.tensor_scalar(
                out=mtmp[:], in0=m0t[:], scalar1=NEG, scalar2=None, op0=OP.mult
            )
            nc.vector.tensor_tensor(out=m0t[:], in0=m0t[:], in1=mtmp[:], op=OP.max)
            nc.vector.tensor_copy(out=m0tb[:], in_=m0t[:])

            qn = [0]

            def gather(out_ap, half_ap, b0, c, elem):
                nc.gpsimd.dma_gather(
                    out_ap=out_ap,
                    in_ap=half_ap,
                    idxs_ap=idx_s[:, 8 * b0 : 8 * (b0 + c)],
                    num_idxs=128 * c,
                    num_idxs_reg=128 * c,
                    elem_size=elem,
                    single_packet=False,
                    queue_num=qn[0] % 4,
                )
                qn[0] += 1

            # ---------------- shared aggregation loop body
            def agg_layer(g, table, row, adst_s, mtb, gp, mtp, ep, accp, saccp, edstp, wp):
                calls = plan_by_group[g]
                nblocks = cblk[g]
                accps = accp.tile([128, 512], f32, tag="acc")
                saccps = saccp.tile([128, 4], f32, tag="sacc")
                bi = 0
                for h, b0, c in calls:
                    xg = gp.tile([128, CJ, row], bf16, tag="xg")
                    gather(
                        xg[:, 0:c, :],
                        table[0:HALFA, :] if h == 0 else table[HALFA:NPOS, :],
                        b0, c, row,
                    )
                    mtile = mtp.tile([128, CJ, 128], bf16, tag="m")
                    nc.sync.dma_start(
                        out=mtile[:, 0:c, :],
                        in_=m_d[:, 128 * b0 : 128 * (b0 + c)],
                    )
                    mttile = mtp.tile([128, CJ, 128], bf16, tag="mt")
                    nc.sync.dma_start(
                        out=mttile[:, 0:c, :],
                        in_=mt_d[:, 128 * b0 : 128 * (b0 + c)],
                    )
                    edst_ps = edstp.tile([128, CJ, 4], f32, tag="edst")
                    for j in range(c):
                        nc.tensor.matmul(
                            edst_ps[:, j, :],
                            lhsT=mttile[:, j, :],
                            rhs=adst_s[:, 4 * g : 4 * (g + 1)],
                            start=True, stop=True,
                        )
                    edst_sb = ep.tile([128, CJ, 4], bf16, tag="edsb")
                    nc.scalar.copy(out=edst_sb[:, 0:c, :], in_=edst_ps[:, 0:c, :])
                    asoff = 128 if row == ROW0 else 512
                    et = ep.tile([128, CJ, 4], bf16, tag="et")
                    nc.vector.tensor_tensor(
                        out=et[:, 0:c, :], in0=xg[:, 0:c, asoff : asoff + 4],
                        in1=edst_sb[:, 0:c, :], op=OP.add,
                    )
                    lr = ep.tile([128, CJ, 4], bf16, tag="lr")
                    nc.vector.tensor_scalar(
                        out=lr[:, 0:c, :], in0=et[:, 0:c, :], scalar1=NEG,
                        scalar2=None, op0=OP.mult,
                    )
                    nc.vector.tensor_tensor(
                        out=et[:, 0:c, :], in0=et[:, 0:c, :], in1=lr[:, 0:c, :], op=OP.max
                    )
                    nc.vector.tensor_tensor(
                        out=et[:, 0:c, :], in0=et[:, 0:c, :],
                        in1=mtb[:, None, :].to_broadcast([128, c, 4]), op=OP.subtract,
                    )
                    nc.scalar.activation(et[:, 0:c, :], et[:, 0:c, :], AF.Exp)
                    wt = ep.tile([128, CJ, 4], f32, tag="wt")
                    nc.vector.tensor_tensor(
                        out=wt[:, 0:c, :], in0=et[:, 0:c, :],
                        in1=ewk_s[:, b0 : b0 + c, None].to_broadcast([128, c, 4]),
                        op=OP.mult,
                    )
                    j = 0
                    while j < c:
                        jw = min(2, c - j)
                        wx = wp.tile([128, 2, 512], bf16, tag="wx")
                        if row == ROW0:
                            in0 = xg[:, j : j + jw, None, 0:128].to_broadcast(
                                [128, jw, 4, 128]
                            )
                        else:
                            in0 = xg[:, j : j + jw, 0:512].rearrange(
                                "p j (h f) -> p j h f", h=4
                            )
                        nc.vector.tensor_tensor(
                            out=wx[:, 0:jw, :].rearrange("p j (h f) -> p j h f", h=4),
                            in0=in0,
                            in1=wt[:, j : j + jw, :, None].to_broadcast(
                                [128, jw, 4, 128]
                            ),
                            op=OP.mult,
                        )
                        for jj in range(jw):
                            nc.tensor.matmul(
                                accps[:], lhsT=mtile[:, j + jj, :], rhs=wx[:, jj, :],
                                start=(bi == 0), stop=(bi == nblocks - 1),
                                skip_group_check=True,
                            )
                            nc.tensor.matmul(
                                saccps[:], lhsT=mtile[:, j + jj, :], rhs=et[:, j + jj, :],
                                start=(bi == 0), stop=(bi == nblocks - 1),
                                skip_group_check=True,
                            )
                            bi += 1
                        j += jw
                return accps, saccps

            # ---------------- L0 + chunked AllGather
            with nc.named_scope("l0"), tc.tile_pool(name="l0g", bufs=3) as gp0, tc.tile_pool(
                name="l0m", bufs=2
            ) as mtp0, tc.tile_pool(name="l0e", bufs=4) as ep0, tc.tile_pool(
                name="l0w", bufs=6
            ) as wp0, tc.tile_pool(name="l0sp", bufs=2) as sp0, tc.tile_pool(
                name="l0acc", bufs=1, space="PSUM"
            ) as accp0, tc.tile_pool(
                name="l0sa", bufs=1, space="PSUM"
            ) as saccp0, tc.tile_pool(
                name="l0ed", bufs=1, space="PSUM"
            ) as edstp0, tc.tile_pool(
                name="l0out", bufs=1, space="PSUM"
            ) as outp0, tc.tile_pool(
                name="l0a", bufs=1, space="PSUM"
            ) as ap0, tc.tile_pool(
                name="l0t", bufs=2, space="PSUM"
            ) as trp0:
                for g in range(G):
                    accps, saccps = agg_layer(
                        g, table0, ROW0, adst0_s, m0tb,
                        gp0, mtp0, ep0, accp0, saccp0, edstp0, wp0,
                    )
                    srec = sp0.tile([128, 4], f32, tag="srec")
                    nc.vector.tensor_scalar(
                        out=srec[:], in0=saccps[:], scalar1=1e-16, scalar2=None, op0=OP.add
                    )
                    nc.vector.reciprocal(srec[:], srec[:])
                    acc_sb = sp0.tile([128, 512], f32, tag="accsb")
                    nc.scalar.copy(out=acc_sb[:], in_=accps[:])
                    aggT = sp0.tile([128, 512], f32, tag="aggT")
                    for hh in range(4):
                        trp = trp0.tile([128, 128], f32, tag="tr", name=f"tr0_{g}_{hh}")
                        nc.tensor.transpose(
                            out=trp[:], in_=acc_sb[:, 128 * hh : 128 * (hh + 1)],
                            identity=ident[:],
                        )
                        nc.scalar.copy(out=aggT[:, 128 * hh : 128 * (hh + 1)], in_=trp[:])
                    outps = outp0.tile([128, 512], f32, tag="out0")
                    for hh in range(4):
                        nc.tensor.matmul(
                            outps[:, 128 * hh : 128 * (hh + 1)],
                            lhsT=aggT[:, 128 * hh : 128 * (hh + 1)],
                            rhs=lin0_s[:, 128 * hh : 128 * (hh + 1)],
                            start=True, stop=True,
                        )
                    h1 = sp0.tile([128, 512], f32, tag="h1")
                    for hh in range(4):
                        nc.scalar.activation(
                            h1[:, 128 * hh : 128 * (hh + 1)],
                            outps[:, 128 * hh : 128 * (hh + 1)],
                            AF.Copy, scale=srec[:, hh : hh + 1],
                        )
                    nc.vector.tensor_tensor(out=h1[:], in0=h1[:], in1=bias0r, op=OP.add)
                    u = sp0.tile([128, 512], f32, tag="elu_u")
                    nc.vector.tensor_scalar(
                        out=u[:], in0=h1[:], scalar1=0.0, scalar2=None, op0=OP.min
                    )
                    v = sp0.tile([128, 512], f32, tag="elu_v")
                    nc.scalar.activation(v[:], u[:], AF.Exp)
                    nc.vector.tensor_tensor(out=h1[:], in0=h1[:], in1=u[:], op=OP.subtract)
                    nc.vector.tensor_tensor(out=h1[:], in0=h1[:], in1=v[:], op=OP.add)
                    nc.vector.tensor_scalar(
                        out=h1[:], in0=h1[:], scalar1=1.0, scalar2=None, op0=OP.subtract
                    )
                    h1T = sp0.tile([128, 512], f32, tag="h1T")
                    for cc in range(4):
                        trp = trp0.tile([128, 128], f32, tag="tr", name=f"trh1_{g}_{cc}")
                        nc.tensor.transpose(
                            out=trp[:], in_=h1[:, 128 * cc : 128 * (cc + 1)],
                            identity=ident[:],
                        )
                        nc.scalar.copy(out=h1T[:, 128 * cc : 128 * (cc + 1)], in_=trp[:])
                    x1ps = outp0.tile([128, 512], f32, tag="x1")
                    aps = ap0.tile([128, 8], f32, tag="aps")
                    for cc in range(4):
                        nc.tensor.matmul(
                            x1ps[:], lhsT=h1T[:, 128 * cc : 128 * (cc + 1)],
                            rhs=lin1_s[cc][:], start=(cc == 0), stop=(cc == 3),
                        )
                    for cc in range(4):
                        nc.tensor.matmul(
                            aps[:], lhsT=h1T[:, 128 * cc : 128 * (cc + 1)],
                            rhs=wa1_s[cc][:], start=(cc == 0), stop=(cc == 3),
                        )
                    stg = sp0.tile([128, ROW1], bf16, tag="stg1")
                    nc.scalar.copy(out=stg[:, 0:512], in_=x1ps[:])
                    nc.vector.tensor_copy(out=stg[:, 512:516], in_=aps[:, 0:4])
                    nc.vector.tensor_copy(
                        out=adst1_s[:, 4 * g : 4 * (g + 1)], in_=aps[:, 4:8]
                    )
                    nc.vector.tensor_tensor(
                        out=amax[:, 8:12], in0=amax[:, 8:12], in1=aps[:, 0:4], op=OP.max
                    )
                    nc.vector.tensor_tensor(
                        out=amax[:, 12:16], in0=amax[:, 12:16], in1=aps[:, 4:8], op=OP.max
                    )
                    nc.sync.dma_start(out=agin[128 * g : 128 * (g + 1), :], in_=stg[:])
                    if g % GPC == GPC - 1:
                        ch = g // GPC
                        with nc.named_scope(f"ag{ch}"):
                            nc.gpsimd.collective_compute(
                                "AllGather",
                                mybir.AluOpType.bypass,
                                replica_groups=[list(range(R))],
                                ins=[agin[GPC * 128 * ch : GPC * 128 * (ch + 1), :].opt()],
                                outs=[table1[CROWS * ch : CROWS * (ch + 1), :].opt()],
                            )

            nc.gpsimd.partition_all_reduce(
                amax[:, 8:16], amax[:, 8:16], 128, bass_isa.ReduceOp.max
            )
            nc.vector.tensor_tensor(
                out=m1t[:], in0=amax[:, 8:12], in1=amax[:, 12:16], op=OP.add
            )
            nc.vector.tensor_scalar(
                out=mtmp[:], in0=m1t[:], scalar1=NEG, scalar2=None, op0=OP.mult
            )
            nc.vector.tensor_tensor(out=m1t[:], in0=m1t[:], in1=mtmp[:], op=OP.max)
            nc.vector.tensor_copy(out=m1tb[:], in_=m1t[:])

            # ---------------- L1 + final MLP
            with nc.named_scope("l1"), tc.tile_pool(name="l1g", bufs=3) as gp1, tc.tile_pool(
                name="l1m", bufs=2
            ) as mtp1, tc.tile_pool(name="l1e", bufs=4) as ep1, tc.tile_pool(
                name="l1w", bufs=6
            ) as wp1p, tc.tile_pool(name="l1sp", bufs=2) as sp1, tc.tile_pool(
                name="l1acc", bufs=2, space="PSUM"
            ) as accp1, tc.tile_pool(
                name="l1sa", bufs=1, space="PSUM"
            ) as saccp1, tc.tile_pool(
                name="l1ed", bufs=1, space="PSUM"
            ) as edstp1, tc.tile_pool(
                name="l1z", bufs=1, space="PSUM"
            ) as zp1, tc.tile_pool(
                name="l1t", bufs=2, space="PSUM"
            ) as trp1:
                for g in range(G):
                    accps, saccps = agg_layer(
                        g, table1, ROW1, adst1_s, m1tb,
                        gp1, mtp1, ep1, accp1, saccp1, edstp1, wp1p,
                    )
                    srec = sp1.tile([128, 4], f32, tag="srec")
                    nc.vector.tensor_scalar(
                        out=srec[:], in0=saccps[:], scalar1=1e-16, scalar2=None, op0=OP.add
                    )
                    nc.vector.reciprocal(srec[:], srec[:])
                    h2 = sp1.tile([128, 512], f32, tag="h2")
                    for hh in range(4):
                        nc.scalar.activation(
                            h2[:, 128 * hh : 128 * (hh + 1)],
                            accps[:, 128 * hh : 128 * (hh + 1)],
                            AF.Copy, scale=srec[:, hh : hh + 1],
                        )
                    nc.vector.tensor_tensor(out=h2[:], in0=h2[:], in1=bias1r, op=OP.add)
                    u = sp1.tile([128, 512], f32, tag="elu_u")
                    nc.vector.tensor_scalar(
                        out=u[:], in0=h2[:], scalar1=0.0, scalar2=None, op0=OP.min
                    )
                    v = sp1.tile([128, 512], f32, tag="elu_v")
                    nc.scalar.activation(v[:], u[:], AF.Exp)
                    nc.vector.tensor_tensor(out=h2[:], in0=h2[:], in1=u[:], op=OP.subtract)
                    nc.vector.tensor_tensor(out=h2[:], in0=h2[:], in1=v[:], op=OP.add)
                    nc.vector.tensor_scalar(
                        out=h2[:], in0=h2[:], scalar1=1.0, scalar2=None, op0=OP.subtract
                    )
                    hT = sp1.tile([128, 640], f32, tag="hT")
                    for cc in range(4):
                        trp = trp1.tile([128, 128], f32, tag="tr", name=f"trh2_{g}_{cc}")
                        nc.tensor.transpose(
                            out=trp[:], in_=h2[:, 128 * cc : 128 * (cc + 1)],
                            identity=ident[:],
                        )
                        nc.scalar.copy(out=hT[:, 128 * cc : 128 * (cc + 1)], in_=trp[:])
                    hrb = sp1.tile([128, 128], f32, tag="hrb")
                    nc.sync.dma_start(out=hrb[:], in_=h_res_d[g, :, :])
                    trp = trp1.tile([128, 128], f32, tag="tr", name=f"trhr_{g}")
                    nc.tensor.transpose(out=trp[:], in_=hrb[:], identity=ident[:])
                    nc.scalar.copy(out=hT[:, 512:640], in_=trp[:])
                    zps = zp1.tile([128, 128], f32, tag="z")
                    for cc in range(5):
                        nc.tensor.matmul(
                            zps[:], lhsT=hT[:, 128 * cc : 128 * (cc + 1)],
                            rhs=wp1_s[cc][:], start=(cc == 0), stop=(cc == 4),
                        )
                    zsb = sp1.tile([128, 128], f32, tag="zsb")
                    nc.scalar.copy(out=zsb[:], in_=zps[:])
                    nc.vector.tensor_tensor(out=zsb[:], in0=zsb[:], in1=bp1r, op=OP.add)
                    nc.scalar.activation(zsb[:], zsb[:], AF.Gelu_apprx_tanh)
                    nc.vector.tensor_tensor(out=zsb[:], in0=zsb[:], in1=wp2r, op=OP.mult)
                    yred = sp1.tile([128, 1], f32, tag="yred")
                    nc.vector.tensor_reduce(yred[:], zsb[:], axis=AX.X, op=OP.add)
                    nc.vector.tensor_scalar(
                        out=ysb[:, g : g + 1], in0=yred[:], scalar1=bp2t[:, 0:1],
                        scalar2=None, op0=OP.add,
                    )
            nc.sync.dma_start(out=y_d[:, :], in_=ysb[:])

    nc.compile()
    return nc


_CACHE = {}


def kernel(
    X,
    edge_index,
    edge_weight,
    W_in,
    b_in,
    lin0,
    att_src0,
    att_dst0,
    bias0,
    lin1,
    att_src1,
    att_dst1,
    bias1,
    Wp1,
    bp1,
    Wp2,
    bp2,
):
    from concourse.bass_utils import run_bass_kernel_spmd

    X = np.asarray(X, np.float32)
    perm, idx16, Mv, Mt, ewk, meta = _prep_graph(
        np.asarray(edge_index), np.asarray(edge_weight)
    )
    rhs0b, rhs0lb, wa1 = _prep_weights(
        np.asarray(W_in, np.float32),
        np.asarray(b_in, np.float32),
        np.asarray(lin0, np.float32),
        np.asarray(att_src0, np.float32),
        np.asarray(att_dst0, np.float32),
        np.asarray(lin1, np.float32),
        np.asarray(att_src1, np.float32),
        np.asarray(att_dst1, np.float32),
    )
    xptb = _build_xpt_blocks(X, perm)

    key = ("nc", meta["nblk"], meta["call_plan"], meta["cblk"])
    if key not in _CACHE:
        _CACHE[key] = _build_nc(list(meta["call_plan"]), list(meta["cblk"]), meta["nblk"])
    nc = _CACHE[key]

    brow = np.zeros((1, 1280), np.float32)
    brow[0, 0:512] = np.asarray(bias0, np.float32)
    brow[0, 512:1024] = np.asarray(bias1, np.float32)
    brow[0, 1024:1152] = np.asarray(bp1, np.float32)
    brow[0, 1152:1280] = np.asarray(Wp2, np.float32)[:, 0]

    common = dict(
        xptb=xptb,
        rhs0=rhs0b,
        rhs0l=rhs0lb,
        lin0=np.ascontiguousarray(np.asarray(lin0, np.float32)),
        lin1=np.ascontiguousarray(np.asarray(lin1, np.float32)),
        wa1=wa1,
        wp1=np.ascontiguousarray(np.asarray(Wp1, np.float32)),
        brow=brow,
        bp2=np.asarray(bp2, np.float32).reshape(1, 1),
    )
    in_maps = [
        dict(
            common,
            xptl=np.ascontiguousarray(xptb[:, G * 128 * r : G * 128 * (r + 1)]),
            idx16=idx16[r],
            ewk=ewk[r],
            md=Mv[r],
            mtd=Mt[r],
        )
        for r in range(R)
    ]

    res = run_bass_kernel_spmd(nc, in_maps, core_ids=list(range(R)), trace=False)

    y = np.zeros(N, np.float32)
    for r in range(R):
        yflat = res.results[r]["y"].T.reshape(-1)
        y[perm[r]] = yflat[: perm[r].shape[0]]
    return y
